# revision 1
# baseline (speedup 1.0000x reference)
"""Trainium2 Bass kernel for nn_Encoder_59219009077683 (Swin-style encoder block).

Mathematical shortcut: the reference's attention einsum 'bhqk,bhqd->bhqd'
multiplies v by the softmax row-sum, which is exactly 1, so the whole
QK/bias/mask/softmax pipeline reduces to o = v followed by a (q,h,d)->(h,q,d)
permutation (the "missing head transpose" in the source model).

Layout insight that makes the permutation free: with head-dim d on partitions
and the flat index j on columns, j = 49*h + q = 12*n + co means the
(h,q)-major and (n,co)-major flattenings coincide, so v and the permuted o
are the *same* tensor ("L_d" layout [32 d, windows*588 j]).

Pipeline per core (4 images, data-parallel over 8 cores):
  gather shifted windows (HBM->SBUF, token-major) -> PE-transpose to
  channel-major -> v-projection (full matmuls) -> 12 partition-block-copy
  DMAs into L_d -> out-projection reading L_d with K=32 strided-column
  matmuls -> LayerNorm1 (partition sums via ones-matmul) + skip ->
  MLP (gelu) -> LayerNorm2 + skip -> PE-transpose back -> scatter to HBM.

Matmuls run in float32r (full PE rate for free-dim >= 256).
"""
import numpy as np
from contextlib import ExitStack

import concourse.bass as bass
import concourse.bacc as bacc
import concourse.tile as tile
from concourse import mybir
from concourse.bass_utils import run_bass_kernel_spmd
from concourse.masks import make_identity

F32 = mybir.dt.float32
F32R = mybir.dt.float32r
AF = mybir.ActivationFunctionType
OP = mybir.AluOpType

B, HH, WW, C = 32, 56, 56, 384
NH, HD, WS, DISP, MLP = 12, 32, 7, 3, 1536
NWS = 8          # windows per side
NWIN = 64        # windows per image
N = 49           # tokens per window
J = NH * N       # 588 flat blocks per window
NCORES = 8
IMGS = B // NCORES
WT = 8           # windows per col-tile
TW = WT * N      # 392
NTILES = NWIN // WT   # 8 col-tiles per image
EPS = 1e-5

WNAMES = ["qkv_w", "qkv_b", "out_w", "out_b", "norm1_g", "norm1_b",
          "norm2_g", "norm2_b", "mlp_w1", "mlp_b1", "mlp_w2", "mlp_b2"]


def _axis_boxes(wi):
    """Window index -> list of (spatial_start, q0, nq) along one axis (roll by DISP)."""
    if wi < NWS - 1:
        return [(WS * wi + DISP, 0, WS)]
    return [(WS * wi + DISP, 0, WS - DISP), (0, WS - DISP, DISP)]


def _ap(t, offset, dims):
    tt = t.tensor if hasattr(t, "tensor") else t
    return bass.AP(tensor=tt, offset=offset, ap=[list(d) for d in dims])


def build():
    nc = bacc.Bacc("TRN2", target_bir_lowering=False, debug=False, num_devices=NCORES)
    x_d = nc.dram_tensor("x", [IMGS, HH, WW, C], F32, kind="ExternalInput")
    qkv_w = nc.dram_tensor("qkv_w", [C, 3 * C], F32, kind="ExternalInput")
    qkv_b = nc.dram_tensor("qkv_b", [3 * C], F32, kind="ExternalInput")
    out_w = nc.dram_tensor("out_w", [C, C], F32, kind="ExternalInput")
    out_b = nc.dram_tensor("out_b", [C], F32, kind="ExternalInput")
    n1g = nc.dram_tensor("norm1_g", [C], F32, kind="ExternalInput")
    n1b = nc.dram_tensor("norm1_b", [C], F32, kind="ExternalInput")
    n2g = nc.dram_tensor("norm2_g", [C], F32, kind="ExternalInput")
    n2b = nc.dram_tensor("norm2_b", [C], F32, kind="ExternalInput")
    w1_d = nc.dram_tensor("mlp_w1", [C, MLP], F32, kind="ExternalInput")
    b1_d = nc.dram_tensor("mlp_b1", [MLP], F32, kind="ExternalInput")
    w2_d = nc.dram_tensor("mlp_w2", [MLP, C], F32, kind="ExternalInput")
    b2_d = nc.dram_tensor("mlp_b2", [C], F32, kind="ExternalInput")
    out_d = nc.dram_tensor("out", [IMGS, HH, WW, C], F32, kind="ExternalOutput")

    with tile.TileContext(nc) as tc, ExitStack() as ctx:
        wpool = ctx.enter_context(tc.tile_pool(name="w", bufs=1))
        stage_pool = ctx.enter_context(tc.tile_pool(name="stage", bufs=2))
        xt_pool = ctx.enter_context(tc.tile_pool(name="xt", bufs=1))
        vt_pool = ctx.enter_context(tc.tile_pool(name="vt", bufs=1))
        ld_pool = ctx.enter_context(tc.tile_pool(name="ld", bufs=1))
        y_pool = ctx.enter_context(tc.tile_pool(name="y", bufs=1))
        x2_pool = ctx.enter_context(tc.tile_pool(name="x2", bufs=1))
        h_pool = ctx.enter_context(tc.tile_pool(name="h", bufs=1))
        oc_pool = ctx.enter_context(tc.tile_pool(name="oc", bufs=1))
        ot_pool = ctx.enter_context(tc.tile_pool(name="ot", bufs=2))
        sm_pool = ctx.enter_context(tc.tile_pool(name="sm", bufs=2))
        ps_big = ctx.enter_context(tc.tile_pool(name="psb", bufs=3, space="PSUM"))
        ps_m = ctx.enter_context(tc.tile_pool(name="psm", bufs=2, space="PSUM"))
        ps_s = ctx.enter_context(tc.tile_pool(name="pss", bufs=3, space="PSUM"))

        # ---------- one-time weight setup ----------
        wv_r = wpool.tile([128, 3 * C], F32R)       # lhsT chunks of Wv
        wo2 = wpool.tile([32, 12 * C], F32R)        # W_out sliced in 12 K=32 blocks
        w1_r = wpool.tile([128, 3 * MLP], F32R)
        w2_r = wpool.tile([128, 12 * C], F32R)
        ones_r = wpool.tile([128, 128], F32R)
        ident = wpool.tile([128, 128], F32)
        eps_t = wpool.tile([128, 1], F32)
        bv_sb = wpool.tile([128, 3], F32)
        bo_sb = wpool.tile([128, 3], F32)
        b1_sb = wpool.tile([128, 12], F32)
        b2_sb = wpool.tile([128, 3], F32)
        g1_sb = wpool.tile([128, 3], F32)
        be1_sb = wpool.tile([128, 3], F32)
        g2_sb = wpool.tile([128, 3], F32)
        be2_sb = wpool.tile([128, 3], F32)

        nc.vector.memset(ident[:, :], 1.0)   # reuse ident tile as f32 scratch
        nc.vector.tensor_copy(ones_r[:, :], ident[:, :])
        make_identity(nc, ident[:, :])
        nc.vector.memset(eps_t[:, :], EPS)
        # PE observes gpsimd's identity once, so later transposes carry <=1 wait
        # (is_transpose lowers to a bare LDWEIGHTS which supports only 1 sync wait)
        dmy0 = ps_m.tile([128, 128], F32, tag="m")
        nc.tensor.transpose(dmy0[:, :], ident[:, :], ident[:, :])

        for k in range(3):
            wtmp = stage_pool.tile([128, MLP], F32, tag="stage")
            nc.sync.dma_start(wtmp[:, 0:C], qkv_w[128 * k:128 * k + 128, 2 * C:3 * C])
            nc.vector.tensor_copy(wv_r[:, C * k:C * k + C], wtmp[:, 0:C])
            wtmp2 = stage_pool.tile([128, MLP], F32, tag="stage")
            nc.sync.dma_start(wtmp2[:, :], w1_d[128 * k:128 * k + 128, :])
            nc.vector.tensor_copy(w1_r[:, MLP * k:MLP * k + MLP], wtmp2[:, :])
        for co in range(12):
            wtmp = stage_pool.tile([128, MLP], F32, tag="stage")
            nc.sync.dma_start(wtmp[0:32, 0:C], out_w[32 * co:32 * co + 32, :])
            nc.vector.tensor_copy(wo2[:, C * co:C * co + C], wtmp[0:32, 0:C])
            wtmp2 = stage_pool.tile([128, MLP], F32, tag="stage")
            nc.sync.dma_start(wtmp2[:, 0:C], w2_d[128 * co:128 * co + 128, :])
            nc.vector.tensor_copy(w2_r[:, C * co:C * co + C], wtmp2[:, 0:C])
        for k in range(3):
            nc.sync.dma_start(bv_sb[:, k:k + 1], qkv_b[2 * C + 128 * k:2 * C + 128 * k + 128])
            nc.sync.dma_start(bo_sb[:, k:k + 1], out_b[128 * k:128 * k + 128])
            nc.sync.dma_start(b2_sb[:, k:k + 1], b2_d[128 * k:128 * k + 128])
            nc.sync.dma_start(g1_sb[:, k:k + 1], n1g[128 * k:128 * k + 128])
            nc.sync.dma_start(be1_sb[:, k:k + 1], n1b[128 * k:128 * k + 128])
            nc.sync.dma_start(g2_sb[:, k:k + 1], n2g[128 * k:128 * k + 128])
            nc.sync.dma_start(be2_sb[:, k:k + 1], n2b[128 * k:128 * k + 128])
        for m in range(12):
            nc.sync.dma_start(b1_sb[:, m:m + 1], b1_d[128 * m:128 * m + 128])

        inv_c = 1.0 / C

        # ---------- main loop ----------
        for img in range(IMGS):
            for quarter in range(4):
                stage = stage_pool.tile([98, 8 * C], F32, tag="stage")
                pst = stage[:, :].ap[0][0]
                soff = stage[:, :].offset
                # gather 16 shifted windows (token-major, 2 windows stacked on partitions)
                for wh in range(16):
                    w_img = 16 * quarter + wh
                    gl, w2i = wh // 2, wh % 2
                    wr, wc = w_img // NWS, w_img % NWS
                    for (r0, qr0, nqr) in _axis_boxes(wr):
                        for (c0, qc0, nqc) in _axis_boxes(wc):
                            if nqc == WS:
                                src = _ap(x_d, ((img * HH + r0) * WW + c0) * C,
                                          [[WW * C, nqr], [C, WS], [1, C]])
                                dst = _ap(stage,
                                          soff + (49 * w2i + WS * qr0) * pst + gl * C,
                                          [[pst, WS * nqr], [1, C]])
                                nc.sync.dma_start(dst, src)
                                continue
                            for i in range(nqr):
                                src = _ap(x_d, ((img * HH + r0 + i) * WW + c0) * C,
                                          [[C, nqc], [1, C]])
                                dst = _ap(stage,
                                          soff + (49 * w2i + WS * (qr0 + i) + qc0) * pst + gl * C,
                                          [[pst, nqc], [1, C]])
                                nc.sync.dma_start(dst, src)

                for tl in range(2):
                    g0 = 4 * tl          # first 2-window group in stage
                    t_img = quarter * 2 + tl
                    w0 = t_img * WT      # first window of tile within image

                    # -- x: token-major -> channel-major (PE transpose) --
                    xt = xt_pool.tile([128, 3 * TW], F32R, tag="xt")
                    for k in range(3):
                        for gg in range(4):
                            xps = ps_m.tile([128, 98], F32, tag="m")
                            nc.tensor.transpose(
                                xps[:, :],
                                stage[0:98, (g0 + gg) * C + 128 * k:(g0 + gg) * C + 128 * k + 128],
                                ident[0:98, 0:98])
                            nc.scalar.copy(xt[:, TW * k + 98 * gg:TW * k + 98 * gg + 98],
                                           xps[:, :])

                    # -- v projection --
                    vt = vt_pool.tile([128, 3 * TW], F32R, tag="vt")
                    for kv in range(3):
                        vps = ps_big.tile([128, TW], F32, tag="big")
                        for k in range(3):
                            nc.tensor.matmul(vps[:, :], wv_r[:, C * k + 128 * kv:C * k + 128 * kv + 128],
                                             xt[:, TW * k:TW * k + TW],
                                             start=(k == 0), stop=(k == 2))
                        nc.scalar.activation(vt[:, TW * kv:TW * kv + TW], vps[:, :],
                                             AF.Identity, bias=bv_sb[:, kv:kv + 1], scale=1.0)

                    # -- permute to L_d layout: 12 partition-block copies --
                    ld = ld_pool.tile([32, WT * J], F32R, tag="ld")
                    pld = ld[:, :].ap[0][0]
                    ldoff = ld[:, :].offset
                    pvt = vt[:, :].ap[0][0]
                    vtoff = vt[:, :].offset
                    for h in range(12):
                        src = _ap(vt, vtoff + ((h % 4) * 32) * pvt + (h // 4) * TW,
                                  [[pvt, 32], [N, WT], [1, N]])
                        dst = _ap(ld, ldoff + N * h,
                                  [[pld, 32], [J, WT], [1, N]])
                        nc.scalar.dma_start(dst, src)

                    # -- out projection: K=32 blocks reading L_d diagonally --
                    y = y_pool.tile([128, 3 * TW], F32R, tag="y")
                    for kj in range(3):
                        yps = ps_big.tile([128, TW], F32, tag="big")
                        for co in range(12):
                            rhs = _ap(ld, ldoff + co, [[pld, 32], [J, WT], [12, N]])
                            nc.tensor.matmul(yps[:, :], wo2[:, C * co + 128 * kj:C * co + 128 * kj + 128],
                                             rhs, start=(co == 0), stop=(co == 11))
                        nc.scalar.activation(y[:, TW * kj:TW * kj + TW], yps[:, :],
                                             AF.Identity, bias=bo_sb[:, kj:kj + 1], scale=1.0)

                    # -- LayerNorm1 + skip -> x2 --
                    s1 = ps_s.tile([128, TW], F32, tag="s")
                    s2 = ps_s.tile([128, TW], F32, tag="s")
                    for k in range(3):
                        ysq = sm_pool.tile([128, TW], F32R, tag="ysq")
                        nc.vector.tensor_mul(ysq[:, :],
                                             y[:, TW * k:TW * k + TW], y[:, TW * k:TW * k + TW])
                        nc.tensor.matmul(s1[:, :], ones_r[:, :], y[:, TW * k:TW * k + TW],
                                         start=(k == 0), stop=(k == 2))
                        nc.tensor.matmul(s2[:, :], ones_r[:, :], ysq[:, :],
                                         start=(k == 0), stop=(k == 2))
                    mu = sm_pool.tile([128, TW], F32, tag="mu")
                    rst = sm_pool.tile([128, TW], F32, tag="rst")
                    tmp = sm_pool.tile([128, TW], F32, tag="tmp")
                    nc.vector.tensor_scalar(mu[:, :], s1[:, :], inv_c, None, op0=OP.mult)
                    nc.vector.tensor_scalar(tmp[:, :], s2[:, :], inv_c, None, op0=OP.mult)
                    nc.vector.tensor_mul(rst[:, :], mu[:, :], mu[:, :])
                    nc.vector.tensor_sub(tmp[:, :], tmp[:, :], rst[:, :])
                    nc.scalar.activation(tmp[:, :], tmp[:, :], AF.Sqrt,
                                         bias=eps_t[:, :], scale=1.0)
                    nc.vector.reciprocal(rst[:, :], tmp[:, :])

                    x2 = x2_pool.tile([128, 3 * TW], F32R, tag="x2")
                    for k in range(3):
                        tk = sm_pool.tile([128, TW], F32, tag="tmp")
                        nc.vector.tensor_sub(tk[:, :], y[:, TW * k:TW * k + TW], mu[:, :])
                        nc.vector.tensor_mul(tk[:, :], tk[:, :], rst[:, :])
                        nc.vector.tensor_scalar(tk[:, :], tk[:, :], g1_sb[:, k:k + 1],
                                                be1_sb[:, k:k + 1], op0=OP.mult, op1=OP.add)
                        nc.vector.tensor_add(x2[:, TW * k:TW * k + TW], tk[:, :],
                                             xt[:, TW * k:TW * k + TW])

                    # -- MLP --
                    hsb = h_pool.tile([128, 12 * TW], F32R, tag="h")
                    for m in range(12):
                        hps = ps_m.tile([128, TW], F32, tag="m")
                        for k in range(3):
                            nc.tensor.matmul(hps[:, :], w1_r[:, MLP * k + 128 * m:MLP * k + 128 * m + 128],
                                             x2[:, TW * k:TW * k + TW],
                                             start=(k == 0), stop=(k == 2))
                        nc.scalar.activation(hsb[:, TW * m:TW * m + TW], hps[:, :],
                                             AF.Gelu, bias=b1_sb[:, m:m + 1], scale=1.0)
                    h2 = y_pool.tile([128, 3 * TW], F32R, tag="h2")
                    for kj in range(3):
                        h2ps = ps_big.tile([128, TW], F32, tag="big")
                        for k2 in range(12):
                            nc.tensor.matmul(h2ps[:, :], w2_r[:, C * k2 + 128 * kj:C * k2 + 128 * kj + 128],
                                             hsb[:, TW * k2:TW * k2 + TW],
                                             start=(k2 == 0), stop=(k2 == 11))
                        nc.scalar.activation(h2[:, TW * kj:TW * kj + TW], h2ps[:, :],
                                             AF.Identity, bias=b2_sb[:, kj:kj + 1], scale=1.0)

                    # -- LayerNorm2 + skip2 -> out (channel-major) --
                    s1b = ps_s.tile([128, TW], F32, tag="s")
                    s2b = ps_s.tile([128, TW], F32, tag="s")
                    for k in range(3):
                        hsq = sm_pool.tile([128, TW], F32R, tag="ysq")
                        nc.vector.tensor_mul(hsq[:, :],
                                             h2[:, TW * k:TW * k + TW], h2[:, TW * k:TW * k + TW])
                        nc.tensor.matmul(s1b[:, :], ones_r[:, :], h2[:, TW * k:TW * k + TW],
                                         start=(k == 0), stop=(k == 2))
                        nc.tensor.matmul(s2b[:, :], ones_r[:, :], hsq[:, :],
                                         start=(k == 0), stop=(k == 2))
                    mu2 = sm_pool.tile([128, TW], F32, tag="mu")
                    rst2 = sm_pool.tile([128, TW], F32, tag="rst")
                    tmp2 = sm_pool.tile([128, TW], F32, tag="tmp")
                    nc.vector.tensor_scalar(mu2[:, :], s1b[:, :], inv_c, None, op0=OP.mult)
                    nc.vector.tensor_scalar(tmp2[:, :], s2b[:, :], inv_c, None, op0=OP.mult)
                    nc.vector.tensor_mul(rst2[:, :], mu2[:, :], mu2[:, :])
                    nc.vector.tensor_sub(tmp2[:, :], tmp2[:, :], rst2[:, :])
                    nc.scalar.activation(tmp2[:, :], tmp2[:, :], AF.Sqrt,
                                         bias=eps_t[:, :], scale=1.0)
                    nc.vector.reciprocal(rst2[:, :], tmp2[:, :])

                    ocm = oc_pool.tile([128, 3 * TW], F32, tag="oc")
                    for k in range(3):
                        tk = sm_pool.tile([128, TW], F32, tag="tmp")
                        nc.vector.tensor_sub(tk[:, :], h2[:, TW * k:TW * k + TW], mu2[:, :])
                        nc.vector.tensor_mul(tk[:, :], tk[:, :], rst2[:, :])
                        nc.vector.tensor_scalar(tk[:, :], tk[:, :], g2_sb[:, k:k + 1],
                                                be2_sb[:, k:k + 1], op0=OP.mult, op1=OP.add)
                        nc.vector.tensor_add(ocm[:, TW * k:TW * k + TW], tk[:, :],
                                             x2[:, TW * k:TW * k + TW])

                    # -- transpose back + scatter to HBM --
                    for gg in range(4):
                        otm = ot_pool.tile([98, C], F32, tag="ot")
                        for k in range(3):
                            ops_t = ps_s.tile([98, 128], F32, tag="s")
                            nc.tensor.transpose(
                                ops_t[:, :],
                                ocm[:, TW * k + 98 * gg:TW * k + 98 * gg + 98],
                                ident[:, :])
                            nc.scalar.copy(otm[:, 128 * k:128 * k + 128], ops_t[:, :])
                        pot = otm[:, :].ap[0][0]
                        otoff = otm[:, :].offset
                        for w2i in range(2):
                            w_img = w0 + 2 * gg + w2i
                            wr, wc = w_img // NWS, w_img % NWS
                            for (r0, qr0, nqr) in _axis_boxes(wr):
                                for (c0, qc0, nqc) in _axis_boxes(wc):
                                    if nqc == WS:
                                        src = _ap(otm, otoff + (49 * w2i + WS * qr0) * pot,
                                                  [[pot, WS * nqr], [1, C]])
                                        dst = _ap(out_d, ((img * HH + r0) * WW + c0) * C,
                                                  [[WW * C, nqr], [C, WS], [1, C]])
                                        nc.sync.dma_start(dst, src)
                                        continue
                                    for i in range(nqr):
                                        src = _ap(otm,
                                                  otoff + (49 * w2i + WS * (qr0 + i) + qc0) * pot,
                                                  [[pot, nqc], [1, C]])
                                        dst = _ap(out_d, ((img * HH + r0 + i) * WW + c0) * C,
                                                  [[C, nqc], [1, C]])
                                        nc.sync.dma_start(dst, src)
    nc.compile()
    return nc


_CACHE = {}


def kernel(**inputs):
    if "nc" not in _CACHE:
        _CACHE["nc"] = build()
    nc = _CACHE["nc"]
    x = np.ascontiguousarray(np.asarray(inputs["x"], dtype=np.float32))
    base = {n: np.ascontiguousarray(np.asarray(inputs[n], dtype=np.float32))
            for n in WNAMES}
    in_maps = []
    for c in range(NCORES):
        m = dict(base)
        m["x"] = np.ascontiguousarray(x[IMGS * c:IMGS * (c + 1)])
        in_maps.append(m)
    import os
    trace = bool(int(os.environ.get("KERNEL_TRACE", "0")))
    res = run_bass_kernel_spmd(nc, in_maps, core_ids=list(range(NCORES)),
                               trace=trace)
    _CACHE["last_res"] = res
    out = np.concatenate([r["out"] for r in res.results], axis=0)
    return out



# revision 10
# speedup vs baseline: 1.3410x; 1.3410x over previous
"""Trainium2 Bass kernel for nn_Encoder_59219009077683 (Swin-style encoder block).

Mathematical shortcut: the reference's attention einsum 'bhqk,bhqd->bhqd'
multiplies v by the softmax row-sum, which is exactly 1, so the whole
QK/bias/mask/softmax pipeline reduces to o = v followed by a (q,h,d)->(h,q,d)
permutation (the "missing head transpose" in the source model).

Layout insight that makes the permutation free: with head-dim d on partitions
and the flat index j on columns, j = 49*h + q = 12*n + co means the
(h,q)-major and (n,co)-major flattenings coincide, so v and the permuted o
are the *same* tensor ("L_d" layout [32 d, windows*588 j]).

Pipeline per core (4 images, data-parallel over 8 cores), processed as 32
"bands" of 8 windows (one shifted window-row, 392 tokens) each:
  gather band with 2 DMAs into [56, 7*C] (partition = shifted column, column
  = row*C+c) -> PE-transpose to channel-major; the PSUM->SBUF copyback APs
  reorder tokens to (window, q)-major -> v-projection -> 12 strided DMAs
  (split over Act/HWDGE and Pool/SWDGE queues) into L_d -> out-projection
  reading L_d with K=32 strided-column matmuls -> LayerNorm1 (partition sums
  via ones-matmul) + skip -> MLP (gelu) -> LayerNorm2 + skip -> PE-transpose
  back per spatial row (strided stationary reads) -> scatter with 2 DMAs.

Intermediates are bf16 (DVE 2x/4x modes, halved SBUF) with f32 PSUM
accumulation; pools are double-buffered so bands pipeline across engines.
"""
import numpy as np
from contextlib import ExitStack

import concourse.bass as bass
import concourse.bacc as bacc
import concourse.tile as tile
from concourse import mybir
from concourse.bass_utils import run_bass_kernel_spmd
from concourse.masks import make_identity

F32 = mybir.dt.float32
BF16 = mybir.dt.bfloat16
AF = mybir.ActivationFunctionType
OP = mybir.AluOpType

B, HH, WW, C = 32, 56, 56, 384
NH, HD, WS, DISP, MLP = 12, 32, 7, 3, 1536
NWS = 8          # windows per side
N = 49           # tokens per window
J = NH * N       # 588 flat blocks per window
NCORES = 8
IMGS = B // NCORES
WT = 8           # windows per band (one shifted window-row)
TW = WT * N      # 392 tokens per band
EPS = 1e-5

WNAMES = ["qkv_w", "qkv_b", "out_w", "out_b", "norm1_g", "norm1_b",
          "norm2_g", "norm2_b", "mlp_w1", "mlp_b1", "mlp_w2", "mlp_b2"]


def _ap(t, offset, dims):
    tt = t.tensor if hasattr(t, "tensor") else t
    return bass.AP(tensor=tt, offset=offset, ap=[list(d) for d in dims])


def _row_runs(band):
    """Shifted row-start -> [(src_row, qr0, n_rows)] for one band."""
    r0 = WS * band + DISP
    if band < NWS - 1:
        return [(r0, 0, WS)]
    return [(r0, 0, WS - DISP), (0, WS - DISP, DISP)]


# Wrap-window (w = 7) column pieces, stored qc-major: (qc0, nqc, src_col).
_WRAP_PIECES = ((0, 4, 52), (4, 3, 0))


def _band_io_dmas(dram, img, band, sb, sb_off, sb_pitch, gather, dma_fn):
    """DMAs moving one shifted band between HBM and a [98, 4*C] tile.
    Partition layout: 49*w2 + 7*qr + qc for the 7 non-wrapping window
    pieces (w2, wp), but 49 + 7*qc + qr for the wrap window (w2, wp) =
    (1, 3) so every DMA's SBUF side is one contiguous partition run."""

    def mk(dst_off, sb_dims, hb_off, hb_dims):
        sb_ap = _ap(sb, sb_off + dst_off, sb_dims)
        hb_ap = _ap(dram, hb_off, hb_dims)
        if gather:
            dma_fn(sb_ap, hb_ap)
        else:
            dma_fn(hb_ap, sb_ap)

    for (sr, qr0, nr) in _row_runs(band):
        base = (img * HH + sr) * WW
        for w2 in range(2):
            for wp in range(4):
                if (w2, wp) == (1, 3):
                    continue
                c0 = 3 + 14 * wp + 7 * w2
                mk((49 * w2 + 7 * qr0) * sb_pitch + wp * C,
                   [[sb_pitch, WS * nr], [1, C]],
                   (base + c0) * C,
                   [[WW * C, nr], [C, WS], [1, C]])
        full = nr == WS
        for (qc0, nqc, c0) in _WRAP_PIECES:
            if full:
                mk((49 + WS * qc0) * sb_pitch + 3 * C,
                   [[sb_pitch, WS * nqc], [1, C]],
                   (base + c0) * C,
                   [[C, nqc], [WW * C, WS], [1, C]])
            else:
                for dqc in range(nqc):
                    mk((49 + WS * (qc0 + dqc) + qr0) * sb_pitch + 3 * C,
                       [[sb_pitch, nr], [1, C]],
                       (base + c0 + dqc) * C,
                       [[WW * C, nr], [1, C]])


def build():
    nc = bacc.Bacc("TRN2", target_bir_lowering=False, debug=False, num_devices=NCORES)
    x_d = nc.dram_tensor("x", [IMGS, HH, WW, C], F32, kind="ExternalInput")
    qkv_w = nc.dram_tensor("qkv_w", [C, 3 * C], F32, kind="ExternalInput")
    qkv_b = nc.dram_tensor("qkv_b", [3 * C], F32, kind="ExternalInput")
    out_w = nc.dram_tensor("out_w", [C, C], F32, kind="ExternalInput")
    out_b = nc.dram_tensor("out_b", [C], F32, kind="ExternalInput")
    n1g = nc.dram_tensor("norm1_g", [C], F32, kind="ExternalInput")
    n1b = nc.dram_tensor("norm1_b", [C], F32, kind="ExternalInput")
    n2g = nc.dram_tensor("norm2_g", [C], F32, kind="ExternalInput")
    n2b = nc.dram_tensor("norm2_b", [C], F32, kind="ExternalInput")
    w1_d = nc.dram_tensor("mlp_w1", [C, MLP], F32, kind="ExternalInput")
    b1_d = nc.dram_tensor("mlp_b1", [MLP], F32, kind="ExternalInput")
    w2_d = nc.dram_tensor("mlp_w2", [MLP, C], F32, kind="ExternalInput")
    b2_d = nc.dram_tensor("mlp_b2", [C], F32, kind="ExternalInput")
    out_d = nc.dram_tensor("out", [IMGS, HH, WW, C], F32, kind="ExternalOutput")

    with tile.TileContext(nc) as tc, ExitStack() as ctx:
        wpool = ctx.enter_context(tc.tile_pool(name="w", bufs=1))
        stage_pool = ctx.enter_context(tc.tile_pool(name="stage", bufs=3))
        xt_pool = ctx.enter_context(tc.tile_pool(name="xt", bufs=2))
        vt_pool = ctx.enter_context(tc.tile_pool(name="vt", bufs=2))
        ld_pool = ctx.enter_context(tc.tile_pool(name="ld", bufs=2))
        y_pool = ctx.enter_context(tc.tile_pool(name="y", bufs=2))
        x2_pool = ctx.enter_context(tc.tile_pool(name="x2", bufs=2))
        h_pool = ctx.enter_context(tc.tile_pool(name="h", bufs=2))
        oc_pool = ctx.enter_context(tc.tile_pool(name="oc", bufs=2))
        xc_pool = ctx.enter_context(tc.tile_pool(name="xc", bufs=2))
        ot_pool = ctx.enter_context(tc.tile_pool(name="ot", bufs=2))
        sm_pool = ctx.enter_context(tc.tile_pool(name="sm", bufs=2))
        ps_t = ctx.enter_context(tc.tile_pool(name="pst", bufs=2, space="PSUM"))
        ps_tb = ctx.enter_context(tc.tile_pool(name="pstb", bufs=1, space="PSUM"))
        ps_big = ctx.enter_context(tc.tile_pool(name="psb", bufs=3, space="PSUM"))
        ps_s = ctx.enter_context(tc.tile_pool(name="pss", bufs=2, space="PSUM"))

        # ---------- one-time weight setup ----------
        wv_r = wpool.tile([128, 3 * C], BF16)       # lhsT chunks of Wv
        wo_r = wpool.tile([128, 3 * C], BF16)       # lhsT chunks of W_out
        w1_r = wpool.tile([128, 3 * MLP], BF16)
        w2_r = wpool.tile([128, 12 * C], BF16)
        ones_b = wpool.tile([128, 128], BF16)
        ident = wpool.tile([128, 128], F32)
        ident_b = wpool.tile([128, 128], BF16)
        eps_t = wpool.tile([128, 1], F32)
        bv_sb = wpool.tile([128, 3], F32)
        bo_sb = wpool.tile([128, 3], F32)
        b1_sb = wpool.tile([128, 12], F32)
        b2_sb = wpool.tile([128, 3], F32)
        g1_sb = wpool.tile([128, 3], F32)
        be1_sb = wpool.tile([128, 3], F32)
        g2_sb = wpool.tile([128, 3], F32)
        be2_sb = wpool.tile([128, 3], F32)

        nc.vector.memset(ones_b[:, :], 1.0)
        make_identity(nc, ident[:, :])
        make_identity(nc, ident_b[:, :])
        nc.vector.memset(eps_t[:, :], EPS)
        # PE observes gpsimd's identity once, so later transposes carry <=1 wait
        # (is_transpose lowers to a bare LDWEIGHTS which supports only 1 sync wait)
        dmy0 = ps_t.tile([128, TW], F32, tag="t")
        nc.tensor.transpose(dmy0[:, 0:128], ident[:, :], ident[:, :])
        dmy1 = ps_tb.tile([98, 3 * 128], BF16, tag="tb")
        nc.tensor.transpose(dmy1[0:98, 0:128], ident_b[:, 0:98], ident_b[:, :])

        for k in range(3):
            wtmp = stage_pool.tile([128, MLP], F32, tag="wst")
            nc.sync.dma_start(wtmp[:, 0:C], qkv_w[128 * k:128 * k + 128, 2 * C:3 * C])
            nc.vector.tensor_copy(wv_r[:, C * k:C * k + C], wtmp[:, 0:C])
            wtmp2 = stage_pool.tile([128, MLP], F32, tag="wst")
            nc.sync.dma_start(wtmp2[:, :], w1_d[128 * k:128 * k + 128, :])
            nc.vector.tensor_copy(w1_r[:, MLP * k:MLP * k + MLP], wtmp2[:, :])
        for k in range(3):
            wtmp = stage_pool.tile([128, MLP], F32, tag="wst")
            nc.sync.dma_start(wtmp[:, 0:C], out_w[128 * k:128 * k + 128, :])
            nc.vector.tensor_copy(wo_r[:, C * k:C * k + C], wtmp[:, 0:C])
        for co in range(12):
            wtmp2 = stage_pool.tile([128, MLP], F32, tag="wst")
            nc.sync.dma_start(wtmp2[:, 0:C], w2_d[128 * co:128 * co + 128, :])
            nc.vector.tensor_copy(w2_r[:, C * co:C * co + C], wtmp2[:, 0:C])
        for k in range(3):
            nc.sync.dma_start(bv_sb[:, k:k + 1], qkv_b[2 * C + 128 * k:2 * C + 128 * k + 128])
            nc.sync.dma_start(bo_sb[:, k:k + 1], out_b[128 * k:128 * k + 128])
            nc.sync.dma_start(b2_sb[:, k:k + 1], b2_d[128 * k:128 * k + 128])
            nc.sync.dma_start(g1_sb[:, k:k + 1], n1g[128 * k:128 * k + 128])
            nc.sync.dma_start(be1_sb[:, k:k + 1], n1b[128 * k:128 * k + 128])
            nc.sync.dma_start(g2_sb[:, k:k + 1], n2g[128 * k:128 * k + 128])
            nc.sync.dma_start(be2_sb[:, k:k + 1], n2b[128 * k:128 * k + 128])
        for m in range(12):
            nc.sync.dma_start(b1_sb[:, m:m + 1], b1_d[128 * m:128 * m + 128])

        inv_c = 1.0 / C

        def layernorm(src, dst, skip, g_sb, be_sb, wrap_fix=False):
            """dst[128,3*TW] = LN_channels(src) * g + be + skip (all bf16)."""
            s1 = ps_s.tile([128, TW], F32, tag="s")
            s2 = ps_s.tile([128, TW], F32, tag="s")
            for k in range(3):
                sq = sm_pool.tile([128, TW], BF16, tag="ysq")
                nc.vector.tensor_mul(sq[:, :], src[:, TW * k:TW * k + TW],
                                     src[:, TW * k:TW * k + TW])
                nc.tensor.matmul(s1[:, :], ones_b[:, :], src[:, TW * k:TW * k + TW],
                                 start=(k == 0), stop=(k == 2))
                nc.tensor.matmul(s2[:, :], ones_b[:, :], sq[:, :],
                                 start=(k == 0), stop=(k == 2))
            mu = sm_pool.tile([128, TW], BF16, tag="mu")
            var = sm_pool.tile([128, TW], BF16, tag="var")
            vsq = sm_pool.tile([128, TW], BF16, tag="vsq")
            std = sm_pool.tile([128, TW], F32, tag="std")
            rst = sm_pool.tile([128, TW], F32, tag="rst")
            rstb = sm_pool.tile([128, TW], BF16, tag="rstb")
            nc.vector.tensor_scalar(mu[:, :], s1[:, :], inv_c, None, op0=OP.mult)
            nc.vector.tensor_scalar(var[:, :], s2[:, :], inv_c, None, op0=OP.mult)
            nc.vector.tensor_mul(vsq[:, :], mu[:, :], mu[:, :])
            nc.vector.tensor_sub(var[:, :], var[:, :], vsq[:, :])
            nc.scalar.activation(std[:, :], var[:, :], AF.Sqrt,
                                 bias=eps_t[:, :], scale=1.0)
            nc.vector.reciprocal(rst[:, :], std[:, :])
            nc.vector.tensor_copy(rstb[:, :], rst[:, :])
            for k in range(3):
                tk = sm_pool.tile([128, TW], BF16, tag="tmp")
                nc.vector.tensor_sub(tk[:, :], src[:, TW * k:TW * k + TW], mu[:, :])
                nc.vector.tensor_mul(tk[:, :], tk[:, :], rstb[:, :])
                nc.vector.tensor_scalar(tk[:, :], tk[:, :], g_sb[:, k:k + 1],
                                        be_sb[:, k:k + 1], op0=OP.mult, op1=OP.add)
                if not wrap_fix:
                    nc.vector.tensor_add(dst[:, TW * k:TW * k + TW], tk[:, :],
                                         skip[:, TW * k:TW * k + TW])
                    continue
                # main run stays (qr, qc)-major; the wrap window's 49 columns
                # are written qc-major so T-out/scatter see contiguous
                # partition runs.
                nc.vector.tensor_add(dst[:, TW * k:TW * k + 343], tk[:, 0:343],
                                     skip[:, TW * k:TW * k + 343])
                ptk = tk[:, :].ap[0][0]
                tkoff = tk[:, :].offset
                psk = skip[:, :].ap[0][0]
                skoff = skip[:, :].offset
                pdst = dst[:, :].ap[0][0]
                dstoff = dst[:, :].offset
                nc.vector.tensor_add(
                    _ap(dst, dstoff + TW * k + 343, [[pdst, 128], [WS, WS], [1, WS]]),
                    _ap(tk, tkoff + 343, [[ptk, 128], [1, WS], [WS, WS]]),
                    _ap(skip, skoff + TW * k + 343, [[psk, 128], [1, WS], [WS, WS]]))

        # ---------- main loop: 4 images x 8 bands, software-pipelined ----------
        # Emission order per step i:  gather(i+2) | B(i)=out-proj+LN1 |
        # A(i+1)=T-in+v-proj+permute+fold | D(i-1)=T-out+scatter | C(i)=MLP+LN2.
        # The A(i+1) PE work covers LN1(i)'s DVE latency; D(i-1) runs after
        # LN2(i-1) has long finished; B(i)'s out-proj finds xcm(i) ready.
        BANDS = [(img, band) for img in range(IMGS) for band in range(NWS)]
        state = {}

        def gather(i):
            img, band = BANDS[i]
            stage = stage_pool.tile([98, 4 * C], F32, tag="stage")
            _band_io_dmas(x_d, img, band, stage, stage[:, :].offset,
                          stage[:, :].ap[0][0], gather=True,
                          dma_fn=nc.sync.dma_start)
            state[("stage", i)] = stage

        def stage_A(i):
            """T-in -> xt; v-projection -> vt; permute -> ld; fold -> xcm."""
            stage = state.pop(("stage", i))
            xt = xt_pool.tile([128, 3 * TW], BF16, tag="xt")
            for k in range(3):
                xps = ps_t.tile([128, TW], F32, tag="t")
                for wp in range(4):
                    nc.tensor.transpose(
                        xps[:, 98 * wp:98 * wp + 98],
                        stage[0:98, wp * C + 128 * k:wp * C + 128 * k + 128],
                        ident[0:98, 0:98])
                pxp = xps[:, :].ap[0][0]
                xpoff = xps[:, :].offset
                pxt = xt[:, :].ap[0][0]
                xtoff = xt[:, :].offset
                nc.scalar.copy(xt[:, TW * k:TW * k + 343], xps[:, 0:343])
                nc.scalar.copy(
                    _ap(xt, xtoff + TW * k + 343, [[pxt, 128], [WS, WS], [1, WS]]),
                    _ap(xps, xpoff + 343, [[pxp, 128], [1, WS], [WS, WS]]))
            vt = vt_pool.tile([128, 3 * TW], BF16, tag="vt")
            for kv in range(3):
                vps = ps_big.tile([128, TW], F32, tag="big")
                for k in range(3):
                    nc.tensor.matmul(vps[:, :],
                                     wv_r[:, C * k + 128 * kv:C * k + 128 * kv + 128],
                                     xt[:, TW * k:TW * k + TW],
                                     start=(k == 0), stop=(k == 2))
                nc.scalar.activation(vt[:, TW * kv:TW * kv + TW], vps[:, :],
                                     AF.Identity, bias=bv_sb[:, kv:kv + 1], scale=1.0)
            ld = ld_pool.tile([32, WT * J], BF16, tag="ld")
            pld = ld[:, :].ap[0][0]
            ldoff = ld[:, :].offset
            pvt = vt[:, :].ap[0][0]
            vtoff = vt[:, :].offset
            for h in range(12):
                src = _ap(vt, vtoff + ((h % 4) * 32) * pvt + (h // 4) * TW,
                          [[pvt, 32], [1, TW]])
                dst = _ap(ld, ldoff + N * h,
                          [[pld, 32], [J, WT], [1, N]])
                if h % 2 == 0:
                    nc.scalar.dma_start(dst, src)
                else:
                    nc.gpsimd.dma_start(dst, src)
            xcm = xc_pool.tile([128, 3 * TW], BF16, tag="xcm")
            pxc = xcm[:, :].ap[0][0]
            xcoff = xcm[:, :].offset
            for kj in range(3):
                for rr in range(4):
                    csrc = _ap(ld, ldoff + 4 * kj + rr,
                               [[pld, 32], [J, WT], [12, N]])
                    cdst = _ap(xcm, xcoff + 32 * rr * pxc + TW * kj,
                               [[pxc, 32], [1, TW]])
                    if rr % 2 == 0:
                        nc.vector.tensor_copy(cdst, csrc)
                    else:
                        nc.gpsimd.tensor_copy(cdst, csrc)
            state[("xt", i)] = xt
            state[("xcm", i)] = xcm

        def stage_B(i):
            """out-projection (K=128) -> y; LayerNorm1 + skip -> x2."""
            xcm = state.pop(("xcm", i))
            y = y_pool.tile([128, 3 * TW], BF16, tag="y")
            for kj in range(3):
                yps = ps_big.tile([128, TW], F32, tag="big")
                for k in range(3):
                    nc.tensor.matmul(yps[:, :],
                                     wo_r[:, C * k + 128 * kj:C * k + 128 * kj + 128],
                                     xcm[:, TW * k:TW * k + TW],
                                     start=(k == 0), stop=(k == 2))
                nc.scalar.activation(y[:, TW * kj:TW * kj + TW], yps[:, :],
                                     AF.Identity, bias=bo_sb[:, kj:kj + 1], scale=1.0)
            x2 = x2_pool.tile([128, 3 * TW], BF16, tag="x2")
            layernorm(y, x2, state.pop(("xt", i)), g1_sb, be1_sb)
            state[("x2", i)] = x2

        def stage_C(i):
            """MLP (gelu) and LayerNorm2 + skip2 -> ocm."""
            x2 = state.pop(("x2", i))
            hsb = h_pool.tile([128, 12 * TW], BF16, tag="h")
            for m in range(12):
                hps = ps_big.tile([128, TW], F32, tag="big")
                for k in range(3):
                    nc.tensor.matmul(hps[:, :],
                                     w1_r[:, MLP * k + 128 * m:MLP * k + 128 * m + 128],
                                     x2[:, TW * k:TW * k + TW],
                                     start=(k == 0), stop=(k == 2))
                nc.scalar.activation(hsb[:, TW * m:TW * m + TW], hps[:, :],
                                     AF.Gelu, bias=b1_sb[:, m:m + 1], scale=1.0)
            h2 = y_pool.tile([128, 3 * TW], BF16, tag="h2")
            for kj in range(3):
                h2ps = ps_big.tile([128, TW], F32, tag="big")
                for k2 in range(12):
                    nc.tensor.matmul(h2ps[:, :],
                                     w2_r[:, C * k2 + 128 * kj:C * k2 + 128 * kj + 128],
                                     hsb[:, TW * k2:TW * k2 + TW],
                                     start=(k2 == 0), stop=(k2 == 11))
                nc.scalar.activation(h2[:, TW * kj:TW * kj + TW], h2ps[:, :],
                                     AF.Identity, bias=b2_sb[:, kj:kj + 1], scale=1.0)
            ocm = oc_pool.tile([128, 3 * TW], BF16, tag="oc")
            layernorm(h2, ocm, x2, g2_sb, be2_sb, wrap_fix=True)
            state[("oc", i)] = ocm

        def stage_D(i):
            """Transpose back and scatter the band."""
            img, band = BANDS[i]
            ocm = state.pop(("oc", i))
            otm = ot_pool.tile([98, 4 * C], F32, tag="ot")
            for wp in range(4):
                otps = ps_tb.tile([98, 3 * 128], BF16, tag="tb")
                for k in range(3):
                    nc.tensor.transpose(
                        otps[0:98, 128 * k:128 * k + 128],
                        ocm[:, TW * k + 98 * wp:TW * k + 98 * wp + 98],
                        ident_b[:, :])
                if wp % 2 == 0:
                    nc.vector.tensor_copy(otm[0:98, wp * C:wp * C + C],
                                          otps[0:98, 0:C])
                else:
                    nc.scalar.copy(otm[0:98, wp * C:wp * C + C],
                                   otps[0:98, 0:C])
            _band_io_dmas(out_d, img, band, otm, otm[:, :].offset,
                          otm[:, :].ap[0][0], gather=False,
                          dma_fn=nc.sync.dma_start)

        NB = len(BANDS)
        gather(0)
        gather(1)
        stage_A(0)
        for i in range(NB):
            if i + 2 < NB:
                gather(i + 2)
            stage_B(i)
            if i + 1 < NB:
                stage_A(i + 1)
            if i > 0:
                stage_D(i - 1)
            stage_C(i)
        stage_D(NB - 1)
    nc.compile()
    return nc


_CACHE = {}


def kernel(**inputs):
    if "nc" not in _CACHE:
        _CACHE["nc"] = build()
    nc = _CACHE["nc"]
    x = np.ascontiguousarray(np.asarray(inputs["x"], dtype=np.float32))
    base = {n: np.ascontiguousarray(np.asarray(inputs[n], dtype=np.float32))
            for n in WNAMES}
    in_maps = []
    for c in range(NCORES):
        m = dict(base)
        m["x"] = np.ascontiguousarray(x[IMGS * c:IMGS * (c + 1)])
        in_maps.append(m)
    import os
    trace = bool(int(os.environ.get("KERNEL_TRACE", "0")))
    res = run_bass_kernel_spmd(nc, in_maps, core_ids=list(range(NCORES)),
                               trace=trace)
    _CACHE["last_res"] = res
    out = np.concatenate([r["out"] for r in res.results], axis=0)
    return out


# revision 22
# speedup vs baseline: 1.4689x; 1.0954x over previous
"""Trainium2 Bass kernel for nn_Encoder_59219009077683 (Swin-style encoder block).

Mathematical shortcut: the reference's attention einsum 'bhqk,bhqd->bhqd'
multiplies v by the softmax row-sum, which is exactly 1, so the whole
QK/bias/mask/softmax pipeline reduces to o = v followed by a (q,h,d)->(h,q,d)
permutation (the "missing head transpose" in the source model).

Layout insight that makes the permutation free: with head-dim d on partitions
and the flat index j on columns, j = 49*h + q = 12*n + co means the
(h,q)-major and (n,co)-major flattenings coincide, so v and the permuted o
are the *same* tensor ("L_d" layout [32 d, windows*588 j]).

Pipeline per core (4 images, data-parallel over 8 cores), processed as 32
"bands" of 8 windows (one shifted window-row, 392 tokens) each:
  gather band with 2 DMAs into [56, 7*C] (partition = shifted column, column
  = row*C+c) -> PE-transpose to channel-major; the PSUM->SBUF copyback APs
  reorder tokens to (window, q)-major -> v-projection -> 12 strided DMAs
  (split over Act/HWDGE and Pool/SWDGE queues) into L_d -> out-projection
  reading L_d with K=32 strided-column matmuls -> LayerNorm1 (partition sums
  via ones-matmul) + skip -> MLP (gelu) -> LayerNorm2 + skip -> PE-transpose
  back per spatial row (strided stationary reads) -> scatter with 2 DMAs.

Intermediates are bf16 (DVE 2x/4x modes, halved SBUF) with f32 PSUM
accumulation; pools are double-buffered so bands pipeline across engines.
"""
import numpy as np
from contextlib import ExitStack

import concourse.bass as bass
import concourse.bacc as bacc
import concourse.tile as tile
from concourse import mybir
from concourse.bass_utils import run_bass_kernel_spmd
from concourse.masks import make_identity

F32 = mybir.dt.float32
BF16 = mybir.dt.bfloat16
AF = mybir.ActivationFunctionType
OP = mybir.AluOpType

B, HH, WW, C = 32, 56, 56, 384
NH, HD, WS, DISP, MLP = 12, 32, 7, 3, 1536
NWS = 8          # windows per side
N = 49           # tokens per window
J = NH * N       # 588 flat blocks per window
NCORES = 8
IMGS = B // NCORES
WT = 8           # windows per band (one shifted window-row)
TW = WT * N      # 392 tokens per band
EPS = 1e-5

WNAMES = ["qkv_w", "qkv_b", "out_w", "out_b", "norm1_g", "norm1_b",
          "norm2_g", "norm2_b", "mlp_w1", "mlp_b1", "mlp_w2", "mlp_b2"]


def _ap(t, offset, dims):
    tt = t.tensor if hasattr(t, "tensor") else t
    return bass.AP(tensor=tt, offset=offset, ap=[list(d) for d in dims])


def _row_runs(band):
    """Shifted row-start -> [(src_row, qr0, n_rows)] for one band."""
    r0 = WS * band + DISP
    if band < NWS - 1:
        return [(r0, 0, WS)]
    return [(r0, 0, WS - DISP), (0, WS - DISP, DISP)]


# Wrap-window (w = 7) column pieces, stored qc-major: (qc0, nqc, src_col).
_WRAP_PIECES = ((0, 4, 52), (4, 3, 0))


def _band_io_dmas(dram, img, band, sb, sb_off, sb_pitch, gather, dma_fn):
    """DMAs moving one shifted band between HBM and a [98, 4*C] tile.
    Partition layout: 49*w2 + 7*qr + qc for the 7 non-wrapping window
    pieces (w2, wp), but 49 + 7*qc + qr for the wrap window (w2, wp) =
    (1, 3) so every DMA's SBUF side is one contiguous partition run."""

    def mk(dst_off, sb_dims, hb_off, hb_dims):
        sb_ap = _ap(sb, sb_off + dst_off, sb_dims)
        hb_ap = _ap(dram, hb_off, hb_dims)
        if gather:
            dma_fn(sb_ap, hb_ap)
        else:
            dma_fn(hb_ap, sb_ap)

    for (sr, qr0, nr) in _row_runs(band):
        base = (img * HH + sr) * WW
        for w2 in range(2):
            for wp in range(4):
                if (w2, wp) == (1, 3):
                    continue
                c0 = 3 + 14 * wp + 7 * w2
                mk((49 * w2 + 7 * qr0) * sb_pitch + wp * C,
                   [[sb_pitch, WS * nr], [1, C]],
                   (base + c0) * C,
                   [[WW * C, nr], [C, WS], [1, C]])
        full = nr == WS
        for (qc0, nqc, c0) in _WRAP_PIECES:
            if full:
                mk((49 + WS * qc0) * sb_pitch + 3 * C,
                   [[sb_pitch, WS * nqc], [1, C]],
                   (base + c0) * C,
                   [[C, nqc], [WW * C, WS], [1, C]])
            else:
                for dqc in range(nqc):
                    mk((49 + WS * (qc0 + dqc) + qr0) * sb_pitch + 3 * C,
                       [[sb_pitch, nr], [1, C]],
                       (base + c0 + dqc) * C,
                       [[WW * C, nr], [1, C]])


def build():
    nc = bacc.Bacc("TRN2", target_bir_lowering=False, debug=False, num_devices=NCORES)
    x_d = nc.dram_tensor("x", [IMGS, HH, WW, C], F32, kind="ExternalInput")
    qkv_w = nc.dram_tensor("qkv_w", [C, 3 * C], F32, kind="ExternalInput")
    qkv_b = nc.dram_tensor("qkv_b", [3 * C], F32, kind="ExternalInput")
    out_w = nc.dram_tensor("out_w", [C, C], F32, kind="ExternalInput")
    out_b = nc.dram_tensor("out_b", [C], F32, kind="ExternalInput")
    n1g = nc.dram_tensor("norm1_g", [C], F32, kind="ExternalInput")
    n1b = nc.dram_tensor("norm1_b", [C], F32, kind="ExternalInput")
    n2g = nc.dram_tensor("norm2_g", [C], F32, kind="ExternalInput")
    n2b = nc.dram_tensor("norm2_b", [C], F32, kind="ExternalInput")
    w1_d = nc.dram_tensor("mlp_w1", [C, MLP], F32, kind="ExternalInput")
    b1_d = nc.dram_tensor("mlp_b1", [MLP], F32, kind="ExternalInput")
    w2_d = nc.dram_tensor("mlp_w2", [MLP, C], F32, kind="ExternalInput")
    b2_d = nc.dram_tensor("mlp_b2", [C], F32, kind="ExternalInput")
    out_d = nc.dram_tensor("out", [IMGS, HH, WW, C], F32, kind="ExternalOutput")

    with tile.TileContext(nc) as tc, ExitStack() as ctx:
        wpool = ctx.enter_context(tc.tile_pool(name="w", bufs=1))
        stage_pool = ctx.enter_context(tc.tile_pool(name="stage", bufs=3))
        xt_pool = ctx.enter_context(tc.tile_pool(name="xt", bufs=2))
        vt_pool = ctx.enter_context(tc.tile_pool(name="vt", bufs=2))
        ld_pool = ctx.enter_context(tc.tile_pool(name="ld", bufs=2))
        y_pool = ctx.enter_context(tc.tile_pool(name="y", bufs=2))
        x2_pool = ctx.enter_context(tc.tile_pool(name="x2", bufs=2))
        h_pool = ctx.enter_context(tc.tile_pool(name="h", bufs=2))
        oc_pool = ctx.enter_context(tc.tile_pool(name="oc", bufs=2))
        xc_pool = ctx.enter_context(tc.tile_pool(name="xc", bufs=2))
        ot_pool = ctx.enter_context(tc.tile_pool(name="ot", bufs=2))
        sm_pool = ctx.enter_context(tc.tile_pool(name="sm", bufs=2))
        ps_t = ctx.enter_context(tc.tile_pool(name="pst", bufs=2, space="PSUM"))
        ps_tb = ctx.enter_context(tc.tile_pool(name="pstb", bufs=1, space="PSUM"))
        ps_big = ctx.enter_context(tc.tile_pool(name="psb", bufs=3, space="PSUM"))
        ps_s = ctx.enter_context(tc.tile_pool(name="pss", bufs=2, space="PSUM"))

        # ---------- one-time weight setup ----------
        wv_r = wpool.tile([128, 3 * C], BF16)       # lhsT chunks of Wv
        wo_r = wpool.tile([128, 3 * C], BF16)       # lhsT chunks of W_out
        w1_r = wpool.tile([128, 3 * MLP], BF16)
        w2_r = wpool.tile([128, 12 * C], BF16)
        ones_b = wpool.tile([128, 128], BF16)
        ident = wpool.tile([128, 128], F32)
        ident_b = wpool.tile([128, 128], BF16)
        eps_t = wpool.tile([128, 1], F32)
        bv_sb = wpool.tile([128, 3], F32)
        bo_sb = wpool.tile([128, 3], F32)
        b1_sb = wpool.tile([128, 12], F32)
        b2_sb = wpool.tile([128, 3], F32)
        g1_sb = wpool.tile([128, 3], F32)
        be1_sb = wpool.tile([128, 3], F32)
        g2_sb = wpool.tile([128, 3], F32)
        be2_sb = wpool.tile([128, 3], F32)

        nc.vector.memset(ones_b[:, :], 1.0)
        make_identity(nc, ident[:, :])
        make_identity(nc, ident_b[:, :])
        nc.vector.memset(eps_t[:, :], EPS)
        # PE observes gpsimd's identity once, so later transposes carry <=1 wait
        # (is_transpose lowers to a bare LDWEIGHTS which supports only 1 sync wait)
        dmy0 = ps_t.tile([128, TW], F32, tag="t")
        nc.tensor.transpose(dmy0[:, 0:128], ident[:, :], ident[:, :])
        dmy1 = ps_tb.tile([98, 3 * 128], BF16, tag="tb")
        nc.tensor.transpose(dmy1[0:98, 0:128], ident_b[:, 0:98], ident_b[:, :])

        for k in range(3):
            wtmp = stage_pool.tile([128, MLP], F32, tag="wst")
            nc.sync.dma_start(wtmp[:, 0:C], qkv_w[128 * k:128 * k + 128, 2 * C:3 * C])
            nc.vector.tensor_copy(wv_r[:, C * k:C * k + C], wtmp[:, 0:C])
            wtmp2 = stage_pool.tile([128, MLP], F32, tag="wst")
            nc.sync.dma_start(wtmp2[:, :], w1_d[128 * k:128 * k + 128, :])
            nc.vector.tensor_copy(w1_r[:, MLP * k:MLP * k + MLP], wtmp2[:, :])
            wtmp3 = stage_pool.tile([128, MLP], F32, tag="wst")
            nc.sync.dma_start(wtmp3[:, 0:C], out_w[128 * k:128 * k + 128, :])
            nc.vector.tensor_copy(wo_r[:, C * k:C * k + C], wtmp3[:, 0:C])
        for co in range(12):
            wtmp4 = stage_pool.tile([128, MLP], F32, tag="wst")
            nc.sync.dma_start(wtmp4[:, 0:C], w2_d[128 * co:128 * co + 128, :])
            nc.vector.tensor_copy(w2_r[:, C * co:C * co + C], wtmp4[:, 0:C])
        for k in range(3):
            nc.sync.dma_start(bv_sb[:, k:k + 1], qkv_b[2 * C + 128 * k:2 * C + 128 * k + 128])
            nc.sync.dma_start(bo_sb[:, k:k + 1], out_b[128 * k:128 * k + 128])
            nc.sync.dma_start(b2_sb[:, k:k + 1], b2_d[128 * k:128 * k + 128])
            nc.sync.dma_start(g1_sb[:, k:k + 1], n1g[128 * k:128 * k + 128])
            nc.sync.dma_start(be1_sb[:, k:k + 1], n1b[128 * k:128 * k + 128])
            nc.sync.dma_start(g2_sb[:, k:k + 1], n2g[128 * k:128 * k + 128])
            nc.sync.dma_start(be2_sb[:, k:k + 1], n2b[128 * k:128 * k + 128])
        for m in range(12):
            nc.sync.dma_start(b1_sb[:, m:m + 1], b1_d[128 * m:128 * m + 128])

        inv_c = 1.0 / C

        def layernorm_pre(src, g_sb, be_sb):
            """ysq + partition sums + mu/var + (src-mu); returns cont state."""
            s1 = ps_s.tile([128, TW], F32, tag="s")
            s2 = ps_s.tile([128, TW], F32, tag="s")
            for k in range(3):
                sq = sm_pool.tile([128, TW], BF16, tag="ysq")
                nc.vector.tensor_mul(sq[:, :], src[:, TW * k:TW * k + TW],
                                     src[:, TW * k:TW * k + TW])
                nc.tensor.matmul(s1[:, :], ones_b[:, :], src[:, TW * k:TW * k + TW],
                                 start=(k == 0), stop=(k == 2))
                nc.tensor.matmul(s2[:, :], ones_b[:, :], sq[:, :],
                                 start=(k == 0), stop=(k == 2))
            mu = sm_pool.tile([128, TW], BF16, tag="mu")
            var = sm_pool.tile([128, TW], BF16, tag="var")
            vsq = sm_pool.tile([128, TW], BF16, tag="vsq")
            nc.vector.tensor_scalar(mu[:, :], s1[:, :], inv_c, None, op0=OP.mult)
            nc.vector.tensor_scalar(var[:, :], s2[:, :], inv_c, None, op0=OP.mult)
            nc.vector.tensor_mul(vsq[:, :], mu[:, :], mu[:, :])
            nc.vector.tensor_sub(var[:, :], var[:, :], vsq[:, :])
            tks = []
            for k in range(3):
                tk = sm_pool.tile([128, TW], BF16, tag="tmp" + str(k))
                nc.vector.tensor_sub(tk[:, :], src[:, TW * k:TW * k + TW], mu[:, :])
                tks.append(tk)
            return var, tks

        def layernorm_post(var, tks, dst, skip, g_sb, be_sb, wrap_fix=False):
            std = sm_pool.tile([128, TW], F32, tag="std")
            rst = sm_pool.tile([128, TW], F32, tag="rst")
            rstb = sm_pool.tile([128, TW], BF16, tag="rstb")
            nc.scalar.activation(std[:, :], var[:, :], AF.Sqrt,
                                 bias=eps_t[:, :], scale=1.0)
            nc.vector.reciprocal(rst[:, :], std[:, :])
            nc.vector.tensor_copy(rstb[:, :], rst[:, :])
            for k in range(3):
                tk = tks[k]
                nc.vector.tensor_mul(tk[:, :], tk[:, :], rstb[:, :])
                nc.vector.tensor_scalar(tk[:, :], tk[:, :], g_sb[:, k:k + 1],
                                        be_sb[:, k:k + 1], op0=OP.mult, op1=OP.add)
                if not wrap_fix:
                    nc.vector.tensor_add(dst[:, TW * k:TW * k + TW], tk[:, :],
                                         skip[:, TW * k:TW * k + TW])
                    continue
                nc.vector.tensor_add(dst[:, TW * k:TW * k + 343], tk[:, 0:343],
                                     skip[:, TW * k:TW * k + 343])
                ptk = tk[:, :].ap[0][0]
                tkoff = tk[:, :].offset
                psk = skip[:, :].ap[0][0]
                skoff = skip[:, :].offset
                pdst = dst[:, :].ap[0][0]
                dstoff = dst[:, :].offset
                nc.vector.tensor_add(
                    _ap(dst, dstoff + TW * k + 343, [[pdst, 128], [WS, WS], [1, WS]]),
                    _ap(tk, tkoff + 343, [[ptk, 128], [1, WS], [WS, WS]]),
                    _ap(skip, skoff + TW * k + 343, [[psk, 128], [1, WS], [WS, WS]]))

        def layernorm(src, dst, skip, g_sb, be_sb, wrap_fix=False):
            var, tks = layernorm_pre(src, g_sb, be_sb)
            layernorm_post(var, tks, dst, skip, g_sb, be_sb, wrap_fix)
            return

        def _dead_layernorm(src, dst, skip, g_sb, be_sb, wrap_fix=False):
            """dst[128,3*TW] = LN_channels(src) * g + be + skip (all bf16)."""
            s1 = ps_s.tile([128, TW], F32, tag="s")
            s2 = ps_s.tile([128, TW], F32, tag="s")
            for k in range(3):
                sq = sm_pool.tile([128, TW], BF16, tag="ysq")
                nc.vector.tensor_mul(sq[:, :], src[:, TW * k:TW * k + TW],
                                     src[:, TW * k:TW * k + TW])
                nc.tensor.matmul(s1[:, :], ones_b[:, :], src[:, TW * k:TW * k + TW],
                                 start=(k == 0), stop=(k == 2))
                nc.tensor.matmul(s2[:, :], ones_b[:, :], sq[:, :],
                                 start=(k == 0), stop=(k == 2))
            mu = sm_pool.tile([128, TW], BF16, tag="mu")
            var = sm_pool.tile([128, TW], BF16, tag="var")
            vsq = sm_pool.tile([128, TW], BF16, tag="vsq")
            std = sm_pool.tile([128, TW], F32, tag="std")
            rst = sm_pool.tile([128, TW], F32, tag="rst")
            rstb = sm_pool.tile([128, TW], BF16, tag="rstb")
            nc.vector.tensor_scalar(mu[:, :], s1[:, :], inv_c, None, op0=OP.mult)
            nc.vector.tensor_scalar(var[:, :], s2[:, :], inv_c, None, op0=OP.mult)
            nc.vector.tensor_mul(vsq[:, :], mu[:, :], mu[:, :])
            nc.vector.tensor_sub(var[:, :], var[:, :], vsq[:, :])
            nc.scalar.activation(std[:, :], var[:, :], AF.Sqrt,
                                 bias=eps_t[:, :], scale=1.0)
            # (src - mu) runs on DVE while Act does the sqrt (+ table load)
            tks = []
            for k in range(3):
                tk = sm_pool.tile([128, TW], BF16, tag="tmp" + str(k))
                nc.vector.tensor_sub(tk[:, :], src[:, TW * k:TW * k + TW], mu[:, :])
                tks.append(tk)
            nc.vector.reciprocal(rst[:, :], std[:, :])
            nc.vector.tensor_copy(rstb[:, :], rst[:, :])
            for k in range(3):
                tk = tks[k]
                nc.vector.tensor_mul(tk[:, :], tk[:, :], rstb[:, :])
                nc.vector.tensor_scalar(tk[:, :], tk[:, :], g_sb[:, k:k + 1],
                                        be_sb[:, k:k + 1], op0=OP.mult, op1=OP.add)
                if not wrap_fix:
                    nc.vector.tensor_add(dst[:, TW * k:TW * k + TW], tk[:, :],
                                         skip[:, TW * k:TW * k + TW])
                    continue
                # main run stays (qr, qc)-major; the wrap window's 49 columns
                # are written qc-major so T-out/scatter see contiguous
                # partition runs.
                nc.vector.tensor_add(dst[:, TW * k:TW * k + 343], tk[:, 0:343],
                                     skip[:, TW * k:TW * k + 343])
                ptk = tk[:, :].ap[0][0]
                tkoff = tk[:, :].offset
                psk = skip[:, :].ap[0][0]
                skoff = skip[:, :].offset
                pdst = dst[:, :].ap[0][0]
                dstoff = dst[:, :].offset
                nc.vector.tensor_add(
                    _ap(dst, dstoff + TW * k + 343, [[pdst, 128], [WS, WS], [1, WS]]),
                    _ap(tk, tkoff + 343, [[ptk, 128], [1, WS], [WS, WS]]),
                    _ap(skip, skoff + TW * k + 343, [[psk, 128], [1, WS], [WS, WS]]))

        # ---------- main loop: 4 images x 8 bands, software-pipelined ----------
        # Emission order per step i:  gather(i+2) | B(i)=out-proj+LN1 |
        # A(i+1)=T-in+v-proj+permute+fold | D(i-1)=T-out+scatter | C(i)=MLP+LN2.
        # The A(i+1) PE work covers LN1(i)'s DVE latency; D(i-1) runs after
        # LN2(i-1) has long finished; B(i)'s out-proj finds xcm(i) ready.
        BANDS = [(img, band) for img in range(IMGS) for band in range(NWS)]
        state = {}

        def gather(i):
            img, band = BANDS[i]
            stage = stage_pool.tile([98, 4 * C], F32, tag="stage")
            _band_io_dmas(x_d, img, band, stage, stage[:, :].offset,
                          stage[:, :].ap[0][0], gather=True,
                          dma_fn=nc.sync.dma_start)
            state[("stage", i)] = stage

        def stage_A(i):
            """T-in -> xt; v-projection -> vt; permute -> ld; fold -> xcm."""
            stage = state.pop(("stage", i))
            xt = xt_pool.tile([128, 3 * TW], BF16, tag="xt")
            for k in range(3):
                xps = ps_t.tile([128, TW], F32, tag="t")
                for wp in range(4):
                    nc.tensor.transpose(
                        xps[:, 98 * wp:98 * wp + 98],
                        stage[0:98, wp * C + 128 * k:wp * C + 128 * k + 128],
                        ident[0:98, 0:98])
                pxp = xps[:, :].ap[0][0]
                xpoff = xps[:, :].offset
                pxt = xt[:, :].ap[0][0]
                xtoff = xt[:, :].offset
                nc.scalar.copy(xt[:, TW * k:TW * k + 343], xps[:, 0:343])
                nc.scalar.copy(
                    _ap(xt, xtoff + TW * k + 343, [[pxt, 128], [WS, WS], [1, WS]]),
                    _ap(xps, xpoff + 343, [[pxp, 128], [1, WS], [WS, WS]]))
            vt = vt_pool.tile([128, 3 * TW], BF16, tag="vt")
            for kv in range(3):
                vps = ps_big.tile([128, TW], F32, tag="big")
                for k in range(3):
                    nc.tensor.matmul(vps[:, :],
                                     wv_r[:, C * k + 128 * kv:C * k + 128 * kv + 128],
                                     xt[:, TW * k:TW * k + TW],
                                     start=(k == 0), stop=(k == 2))
                nc.scalar.activation(vt[:, TW * kv:TW * kv + TW], vps[:, :],
                                     AF.Identity, bias=bv_sb[:, kv:kv + 1], scale=1.0)
            ld = ld_pool.tile([32, WT * J], BF16, tag="ld")
            pld = ld[:, :].ap[0][0]
            ldoff = ld[:, :].offset
            pvt = vt[:, :].ap[0][0]
            vtoff = vt[:, :].offset
            for h in range(12):
                src = _ap(vt, vtoff + ((h % 4) * 32) * pvt + (h // 4) * TW,
                          [[pvt, 32], [1, TW]])
                dst = _ap(ld, ldoff + N * h,
                          [[pld, 32], [J, WT], [1, N]])
                if h % 2 == 0:
                    nc.scalar.dma_start(dst, src)
                else:
                    nc.gpsimd.dma_start(dst, src)
            state[("xt", i)] = xt
            state[("ld", i)] = ld

        def stage_A2(i):
            """Fold L_d -> channel-major xcm (strided copies, DVE/Pool)."""
            ld = state.pop(("ld", i))
            pld = ld[:, :].ap[0][0]
            ldoff = ld[:, :].offset
            xcm = xc_pool.tile([128, 3 * TW], BF16, tag="xcm")
            pxc = xcm[:, :].ap[0][0]
            xcoff = xcm[:, :].offset
            for kj in range(3):
                for rr in range(4):
                    csrc = _ap(ld, ldoff + 4 * kj + rr,
                               [[pld, 32], [J, WT], [12, N]])
                    cdst = _ap(xcm, xcoff + 32 * rr * pxc + TW * kj,
                               [[pxc, 32], [1, TW]])
                    nc.gpsimd.tensor_copy(cdst, csrc)
            state[("xcm", i)] = xcm

        def stage_B_out(i):
            """out-projection (K=128) -> y."""
            xcm = state.pop(("xcm", i))
            y = y_pool.tile([128, 3 * TW], BF16, tag="y")
            for kj in range(3):
                yps = ps_big.tile([128, TW], F32, tag="big")
                for k in range(3):
                    nc.tensor.matmul(yps[:, :],
                                     wo_r[:, C * k + 128 * kj:C * k + 128 * kj + 128],
                                     xcm[:, TW * k:TW * k + TW],
                                     start=(k == 0), stop=(k == 2))
                nc.vector.tensor_scalar(y[:, TW * kj:TW * kj + TW], yps[:, :],
                                        1.0, bo_sb[:, kj:kj + 1],
                                        op0=OP.mult, op1=OP.add)
            state[("y", i)] = y

        def stage_B_ln_pre(i):
            state[("ln1", i)] = layernorm_pre(state.pop(("y", i)), g1_sb, be1_sb)

        def stage_B_ln_post(i):
            var, tks = state.pop(("ln1", i))
            x2 = x2_pool.tile([128, 3 * TW], BF16, tag="x2")
            layernorm_post(var, tks, x2, state.pop(("xt", i)), g1_sb, be1_sb)
            state[("x2", i)] = x2

        def stage_C(i):
            """MLP (gelu) and LayerNorm2 + skip2 -> ocm."""
            x2 = state.pop(("x2", i))
            hsb = h_pool.tile([128, 12 * TW], BF16, tag="h")
            for m in range(12):
                hps = ps_big.tile([128, TW], F32, tag="big")
                for k in range(3):
                    nc.tensor.matmul(hps[:, :],
                                     w1_r[:, MLP * k + 128 * m:MLP * k + 128 * m + 128],
                                     x2[:, TW * k:TW * k + TW],
                                     start=(k == 0), stop=(k == 2))
                nc.scalar.activation(hsb[:, TW * m:TW * m + TW], hps[:, :],
                                     AF.Gelu, bias=b1_sb[:, m:m + 1], scale=1.0)
            h2 = y_pool.tile([128, 3 * TW], BF16, tag="h2")
            for kj in range(3):
                h2ps = ps_big.tile([128, TW], F32, tag="big")
                for k2 in range(12):
                    nc.tensor.matmul(h2ps[:, :],
                                     w2_r[:, C * k2 + 128 * kj:C * k2 + 128 * kj + 128],
                                     hsb[:, TW * k2:TW * k2 + TW],
                                     start=(k2 == 0), stop=(k2 == 11))
                nc.vector.tensor_scalar(h2[:, TW * kj:TW * kj + TW], h2ps[:, :],
                                        1.0, b2_sb[:, kj:kj + 1],
                                        op0=OP.mult, op1=OP.add)
            ocm = oc_pool.tile([128, 3 * TW], BF16, tag="oc")
            layernorm(h2, ocm, x2, g2_sb, be2_sb, wrap_fix=True)
            state[("oc", i)] = ocm

        def stage_D(i):
            """Transpose back and scatter the band."""
            img, band = BANDS[i]
            ocm = state.pop(("oc", i))
            otm = ot_pool.tile([98, 4 * C], F32, tag="ot")
            for wp in range(4):
                otps = ps_tb.tile([98, 3 * 128], BF16, tag="tb")
                for k in range(3):
                    nc.tensor.transpose(
                        otps[0:98, 128 * k:128 * k + 128],
                        ocm[:, TW * k + 98 * wp:TW * k + 98 * wp + 98],
                        ident_b[:, :])
                nc.scalar.copy(otm[0:98, wp * C:wp * C + C],
                               otps[0:98, 0:C])
            _band_io_dmas(out_d, img, band, otm, otm[:, :].offset,
                          otm[:, :].ap[0][0], gather=False,
                          dma_fn=nc.sync.dma_start)

        NB = len(BANDS)
        gather(0)
        gather(1)
        stage_A(0)
        stage_A2(0)
        for i in range(NB):
            if i + 1 < NB:
                gather(i + 1)
            stage_B_out(i)
            if i + 1 < NB:
                stage_A(i + 1)
                stage_A2(i + 1)
            stage_B_ln_pre(i)
            stage_B_ln_post(i)
            if i >= 2:
                stage_D(i - 2)
            if i >= 1:
                stage_C(i - 1)
        stage_C(NB - 1)
        stage_D(NB - 2)
        stage_D(NB - 1)
    nc.compile()
    return nc


_CACHE = {}


def kernel(**inputs):
    if "nc" not in _CACHE:
        _CACHE["nc"] = build()
    nc = _CACHE["nc"]
    x = np.ascontiguousarray(np.asarray(inputs["x"], dtype=np.float32))
    base = {n: np.ascontiguousarray(np.asarray(inputs[n], dtype=np.float32))
            for n in WNAMES}
    in_maps = []
    for c in range(NCORES):
        m = dict(base)
        m["x"] = np.ascontiguousarray(x[IMGS * c:IMGS * (c + 1)])
        in_maps.append(m)
    import os
    trace = bool(int(os.environ.get("KERNEL_TRACE", "0")))
    res = run_bass_kernel_spmd(nc, in_maps, core_ids=list(range(NCORES)),
                               trace=trace)
    _CACHE["last_res"] = res
    out = np.concatenate([r["out"] for r in res.results], axis=0)
    return out


# revision 31
# speedup vs baseline: 1.5120x; 1.0293x over previous
"""Trainium2 Bass kernel for nn_Encoder_59219009077683 (Swin-style encoder block).

Mathematical shortcut: the reference's attention einsum 'bhqk,bhqd->bhqd'
multiplies v by the softmax row-sum, which is exactly 1, so the whole
QK/bias/mask/softmax pipeline reduces to o = v followed by a (q,h,d)->(h,q,d)
permutation (the "missing head transpose" in the source model).

Layout insight that makes the permutation free: with head-dim d on partitions
and the flat index j on columns, j = 49*h + q = 12*n + co means the
(h,q)-major and (n,co)-major flattenings coincide, so v and the permuted o
are the *same* tensor ("L_d" layout [32 d, windows*588 j]).

Pipeline per core (4 images, data-parallel over 8 cores), processed as 32
"bands" of 8 windows (one shifted window-row, 392 tokens) each:
  gather band with 2 DMAs into [56, 7*C] (partition = shifted column, column
  = row*C+c) -> PE-transpose to channel-major; the PSUM->SBUF copyback APs
  reorder tokens to (window, q)-major -> v-projection -> 12 strided DMAs
  (split over Act/HWDGE and Pool/SWDGE queues) into L_d -> out-projection
  reading L_d with K=32 strided-column matmuls -> LayerNorm1 (partition sums
  via ones-matmul) + skip -> MLP (gelu) -> LayerNorm2 + skip -> PE-transpose
  back per spatial row (strided stationary reads) -> scatter with 2 DMAs.

Intermediates are bf16 (DVE 2x/4x modes, halved SBUF) with f32 PSUM
accumulation; pools are double-buffered so bands pipeline across engines.
"""
import numpy as np
from contextlib import ExitStack

import concourse.bass as bass
import concourse.bacc as bacc
import concourse.tile as tile
from concourse import mybir
from concourse.bass_utils import run_bass_kernel_spmd
from concourse.masks import make_identity

F32 = mybir.dt.float32
BF16 = mybir.dt.bfloat16
AF = mybir.ActivationFunctionType
OP = mybir.AluOpType

B, HH, WW, C = 32, 56, 56, 384
NH, HD, WS, DISP, MLP = 12, 32, 7, 3, 1536
NWS = 8          # windows per side
N = 49           # tokens per window
J = NH * N       # 588 flat blocks per window
NCORES = 8
IMGS = B // NCORES
WT = 8           # windows per band (one shifted window-row)
TW = WT * N      # 392 tokens per band
EPS = 1e-5

WNAMES = ["qkv_w", "qkv_b", "out_w", "out_b", "norm1_g", "norm1_b",
          "norm2_g", "norm2_b", "mlp_w1", "mlp_b1", "mlp_w2", "mlp_b2"]


def _ap(t, offset, dims):
    tt = t.tensor if hasattr(t, "tensor") else t
    return bass.AP(tensor=tt, offset=offset, ap=[list(d) for d in dims])


def _row_runs(band):
    """Shifted row-start -> [(src_row, qr0, n_rows)] for one band."""
    r0 = WS * band + DISP
    if band < NWS - 1:
        return [(r0, 0, WS)]
    return [(r0, 0, WS - DISP), (0, WS - DISP, DISP)]


# Wrap-window (w = 7) column pieces, stored qc-major: (qc0, nqc, src_col).
_WRAP_PIECES = ((0, 4, 52), (4, 3, 0))


def _band_io_dmas(dram, img, band, sb, sb_off, sb_pitch, gather, dma_fn):
    """DMAs moving one shifted band between HBM and a [98, 4*C] tile.
    Partition layout: 49*w2 + 7*qr + qc for the 7 non-wrapping window
    pieces (w2, wp), but 49 + 7*qc + qr for the wrap window (w2, wp) =
    (1, 3) so every DMA's SBUF side is one contiguous partition run."""

    def mk(dst_off, sb_dims, hb_off, hb_dims):
        sb_ap = _ap(sb, sb_off + dst_off, sb_dims)
        hb_ap = _ap(dram, hb_off, hb_dims)
        if gather:
            dma_fn(sb_ap, hb_ap)
        else:
            dma_fn(hb_ap, sb_ap)

    for (sr, qr0, nr) in _row_runs(band):
        base = (img * HH + sr) * WW
        for w2 in range(2):
            for wp in range(4):
                if (w2, wp) == (1, 3):
                    continue
                c0 = 3 + 14 * wp + 7 * w2
                mk((49 * w2 + 7 * qr0) * sb_pitch + wp * C,
                   [[sb_pitch, WS * nr], [1, C]],
                   (base + c0) * C,
                   [[WW * C, nr], [C, WS], [1, C]])
        full = nr == WS
        for (qc0, nqc, c0) in _WRAP_PIECES:
            if full:
                mk((49 + WS * qc0) * sb_pitch + 3 * C,
                   [[sb_pitch, WS * nqc], [1, C]],
                   (base + c0) * C,
                   [[C, nqc], [WW * C, WS], [1, C]])
            else:
                for dqc in range(nqc):
                    mk((49 + WS * (qc0 + dqc) + qr0) * sb_pitch + 3 * C,
                       [[sb_pitch, nr], [1, C]],
                       (base + c0 + dqc) * C,
                       [[WW * C, nr], [1, C]])


def build():
    nc = bacc.Bacc("TRN2", target_bir_lowering=False, debug=False, num_devices=NCORES)
    x_d = nc.dram_tensor("x", [IMGS, HH, WW, C], F32, kind="ExternalInput")
    qkv_w = nc.dram_tensor("qkv_w", [C, 3 * C], F32, kind="ExternalInput")
    qkv_b = nc.dram_tensor("qkv_b", [3 * C], F32, kind="ExternalInput")
    out_w = nc.dram_tensor("out_w", [C, C], F32, kind="ExternalInput")
    out_b = nc.dram_tensor("out_b", [C], F32, kind="ExternalInput")
    n1g = nc.dram_tensor("norm1_g", [C], F32, kind="ExternalInput")
    n1b = nc.dram_tensor("norm1_b", [C], F32, kind="ExternalInput")
    n2g = nc.dram_tensor("norm2_g", [C], F32, kind="ExternalInput")
    n2b = nc.dram_tensor("norm2_b", [C], F32, kind="ExternalInput")
    w1_d = nc.dram_tensor("mlp_w1", [C, MLP], F32, kind="ExternalInput")
    b1_d = nc.dram_tensor("mlp_b1", [MLP], F32, kind="ExternalInput")
    w2_d = nc.dram_tensor("mlp_w2", [MLP, C], F32, kind="ExternalInput")
    b2_d = nc.dram_tensor("mlp_b2", [C], F32, kind="ExternalInput")
    out_d = nc.dram_tensor("out", [IMGS, HH, WW, C], F32, kind="ExternalOutput")

    with tile.TileContext(nc) as tc, ExitStack() as ctx:
        wpool = ctx.enter_context(tc.tile_pool(name="w", bufs=1))
        stage_pool = ctx.enter_context(tc.tile_pool(name="stage", bufs=3))
        xt_pool = ctx.enter_context(tc.tile_pool(name="xt", bufs=2))
        vt_pool = ctx.enter_context(tc.tile_pool(name="vt", bufs=2))
        ld_pool = ctx.enter_context(tc.tile_pool(name="ld", bufs=2))
        y_pool = ctx.enter_context(tc.tile_pool(name="y", bufs=2))
        x2_pool = ctx.enter_context(tc.tile_pool(name="x2", bufs=2))
        h_pool = ctx.enter_context(tc.tile_pool(name="h", bufs=2))
        oc_pool = ctx.enter_context(tc.tile_pool(name="oc", bufs=2))
        xc_pool = ctx.enter_context(tc.tile_pool(name="xc", bufs=2))
        ot_pool = ctx.enter_context(tc.tile_pool(name="ot", bufs=2))
        sm_pool = ctx.enter_context(tc.tile_pool(name="sm", bufs=2))
        ps_t = ctx.enter_context(tc.tile_pool(name="pst", bufs=2, space="PSUM"))
        ps_tb = ctx.enter_context(tc.tile_pool(name="pstb", bufs=1, space="PSUM"))
        ps_big = ctx.enter_context(tc.tile_pool(name="psb", bufs=3, space="PSUM"))
        ps_s = ctx.enter_context(tc.tile_pool(name="pss", bufs=2, space="PSUM"))

        # ---------- one-time weight setup ----------
        wv_r = wpool.tile([128, 3 * C], BF16)       # lhsT chunks of Wv
        wo_r = wpool.tile([128, 3 * C], BF16)       # lhsT chunks of W_out
        w1_r = wpool.tile([128, 3 * MLP], BF16)
        w2_r = wpool.tile([128, 12 * C], BF16)
        ones_b = wpool.tile([128, 128], BF16)
        ident = wpool.tile([128, 128], F32)
        ident_b = wpool.tile([128, 128], BF16)
        eps_t = wpool.tile([128, 1], F32)
        bv_sb = wpool.tile([128, 3], F32)
        bo_sb = wpool.tile([128, 3], F32)
        b1_sb = wpool.tile([128, 12], F32)
        b2_sb = wpool.tile([128, 3], F32)
        g1_sb = wpool.tile([128, 3], F32)
        be1_sb = wpool.tile([128, 3], F32)
        g2_sb = wpool.tile([128, 3], F32)
        be2_sb = wpool.tile([128, 3], F32)

        nc.vector.memset(ones_b[:, :], 1.0)
        make_identity(nc, ident[:, :])
        make_identity(nc, ident_b[:, :])
        nc.vector.memset(eps_t[:, :], EPS)
        # PE observes gpsimd's identity once, so later transposes carry <=1 wait
        # (is_transpose lowers to a bare LDWEIGHTS which supports only 1 sync wait)
        dmy0 = ps_t.tile([128, TW], F32, tag="t")
        nc.tensor.transpose(dmy0[:, 0:128], ident[:, :], ident[:, :])
        dmy1 = ps_tb.tile([98, 3 * 128], BF16, tag="tb")
        nc.tensor.transpose(dmy1[0:98, 0:128], ident_b[:, 0:98], ident_b[:, :])

        for k in range(3):
            wtmp = stage_pool.tile([128, MLP], F32, tag="wst")
            nc.sync.dma_start(wtmp[:, 0:C], qkv_w[128 * k:128 * k + 128, 2 * C:3 * C])
            nc.vector.tensor_copy(wv_r[:, C * k:C * k + C], wtmp[:, 0:C])
            wtmp2 = stage_pool.tile([128, MLP], F32, tag="wst")
            nc.sync.dma_start(wtmp2[:, :], w1_d[128 * k:128 * k + 128, :])
            nc.vector.tensor_copy(w1_r[:, MLP * k:MLP * k + MLP], wtmp2[:, :])
            wtmp3 = stage_pool.tile([128, MLP], F32, tag="wst")
            nc.sync.dma_start(wtmp3[:, 0:C], out_w[128 * k:128 * k + 128, :])
            nc.vector.tensor_copy(wo_r[:, C * k:C * k + C], wtmp3[:, 0:C])
        for co in range(12):
            wtmp4 = stage_pool.tile([128, MLP], F32, tag="wst")
            nc.sync.dma_start(wtmp4[:, 0:C], w2_d[128 * co:128 * co + 128, :])
            nc.vector.tensor_copy(w2_r[:, C * co:C * co + C], wtmp4[:, 0:C])
        for k in range(3):
            nc.sync.dma_start(bv_sb[:, k:k + 1], qkv_b[2 * C + 128 * k:2 * C + 128 * k + 128])
            nc.sync.dma_start(bo_sb[:, k:k + 1], out_b[128 * k:128 * k + 128])
            nc.sync.dma_start(b2_sb[:, k:k + 1], b2_d[128 * k:128 * k + 128])
            nc.sync.dma_start(g1_sb[:, k:k + 1], n1g[128 * k:128 * k + 128])
            nc.sync.dma_start(be1_sb[:, k:k + 1], n1b[128 * k:128 * k + 128])
            nc.sync.dma_start(g2_sb[:, k:k + 1], n2g[128 * k:128 * k + 128])
            nc.sync.dma_start(be2_sb[:, k:k + 1], n2b[128 * k:128 * k + 128])
        for m in range(12):
            nc.sync.dma_start(b1_sb[:, m:m + 1], b1_d[128 * m:128 * m + 128])

        inv_c = 1.0 / C

        def layernorm_pre(src, g_sb, be_sb):
            """ysq + partition sums + mu/var + (src-mu); returns cont state."""
            s1 = ps_s.tile([128, TW], F32, tag="s")
            s2 = ps_s.tile([128, TW], F32, tag="s")
            for k in range(3):
                sq = sm_pool.tile([128, TW], BF16, tag="ysq")
                nc.vector.tensor_mul(sq[:, :], src[:, TW * k:TW * k + TW],
                                     src[:, TW * k:TW * k + TW])
                nc.tensor.matmul(s1[:, :], ones_b[:, :], src[:, TW * k:TW * k + TW],
                                 start=(k == 0), stop=(k == 2))
                nc.tensor.matmul(s2[:, :], ones_b[:, :], sq[:, :],
                                 start=(k == 0), stop=(k == 2))
            mu = sm_pool.tile([128, TW], BF16, tag="mu")
            var = sm_pool.tile([128, TW], BF16, tag="var")
            vsq = sm_pool.tile([128, TW], BF16, tag="vsq")
            nc.vector.tensor_scalar(mu[:, :], s1[:, :], inv_c, None, op0=OP.mult)
            nc.vector.tensor_scalar(var[:, :], s2[:, :], inv_c, None, op0=OP.mult)
            nc.vector.tensor_mul(vsq[:, :], mu[:, :], mu[:, :])
            nc.vector.tensor_sub(var[:, :], var[:, :], vsq[:, :])
            tks = []
            for k in range(3):
                tk = sm_pool.tile([128, TW], BF16, tag="tmp" + str(k))
                nc.vector.tensor_sub(tk[:, :], src[:, TW * k:TW * k + TW], mu[:, :])
                tks.append(tk)
            return var, tks

        def layernorm_post(var, tks, dst, skip, g_sb, be_sb, wrap_fix=False):
            std = sm_pool.tile([128, TW], F32, tag="std")
            rst = sm_pool.tile([128, TW], F32, tag="rst")
            rstb = sm_pool.tile([128, TW], BF16, tag="rstb")
            nc.scalar.activation(std[:, :], var[:, :], AF.Sqrt,
                                 bias=eps_t[:, :], scale=1.0)
            nc.vector.reciprocal(rst[:, :], std[:, :])
            nc.vector.tensor_copy(rstb[:, :], rst[:, :])
            for k in range(3):
                tk = tks[k]
                nc.vector.tensor_mul(tk[:, :], tk[:, :], rstb[:, :])
                nc.vector.tensor_scalar(tk[:, :], tk[:, :], g_sb[:, k:k + 1],
                                        be_sb[:, k:k + 1], op0=OP.mult, op1=OP.add)
                if not wrap_fix:
                    nc.vector.tensor_add(dst[:, TW * k:TW * k + TW], tk[:, :],
                                         skip[:, TW * k:TW * k + TW])
                    continue
                nc.vector.tensor_add(dst[:, TW * k:TW * k + 343], tk[:, 0:343],
                                     skip[:, TW * k:TW * k + 343])
                ptk = tk[:, :].ap[0][0]
                tkoff = tk[:, :].offset
                psk = skip[:, :].ap[0][0]
                skoff = skip[:, :].offset
                pdst = dst[:, :].ap[0][0]
                dstoff = dst[:, :].offset
                nc.vector.tensor_add(
                    _ap(dst, dstoff + TW * k + 343, [[pdst, 128], [WS, WS], [1, WS]]),
                    _ap(tk, tkoff + 343, [[ptk, 128], [1, WS], [WS, WS]]),
                    _ap(skip, skoff + TW * k + 343, [[psk, 128], [1, WS], [WS, WS]]))

        def layernorm(src, dst, skip, g_sb, be_sb, wrap_fix=False):
            var, tks = layernorm_pre(src, g_sb, be_sb)
            layernorm_post(var, tks, dst, skip, g_sb, be_sb, wrap_fix)
            return

        def _dead_layernorm(src, dst, skip, g_sb, be_sb, wrap_fix=False):
            """dst[128,3*TW] = LN_channels(src) * g + be + skip (all bf16)."""
            s1 = ps_s.tile([128, TW], F32, tag="s")
            s2 = ps_s.tile([128, TW], F32, tag="s")
            for k in range(3):
                sq = sm_pool.tile([128, TW], BF16, tag="ysq")
                nc.vector.tensor_mul(sq[:, :], src[:, TW * k:TW * k + TW],
                                     src[:, TW * k:TW * k + TW])
                nc.tensor.matmul(s1[:, :], ones_b[:, :], src[:, TW * k:TW * k + TW],
                                 start=(k == 0), stop=(k == 2))
                nc.tensor.matmul(s2[:, :], ones_b[:, :], sq[:, :],
                                 start=(k == 0), stop=(k == 2))
            mu = sm_pool.tile([128, TW], BF16, tag="mu")
            var = sm_pool.tile([128, TW], BF16, tag="var")
            vsq = sm_pool.tile([128, TW], BF16, tag="vsq")
            std = sm_pool.tile([128, TW], F32, tag="std")
            rst = sm_pool.tile([128, TW], F32, tag="rst")
            rstb = sm_pool.tile([128, TW], BF16, tag="rstb")
            nc.vector.tensor_scalar(mu[:, :], s1[:, :], inv_c, None, op0=OP.mult)
            nc.vector.tensor_scalar(var[:, :], s2[:, :], inv_c, None, op0=OP.mult)
            nc.vector.tensor_mul(vsq[:, :], mu[:, :], mu[:, :])
            nc.vector.tensor_sub(var[:, :], var[:, :], vsq[:, :])
            nc.scalar.activation(std[:, :], var[:, :], AF.Sqrt,
                                 bias=eps_t[:, :], scale=1.0)
            # (src - mu) runs on DVE while Act does the sqrt (+ table load)
            tks = []
            for k in range(3):
                tk = sm_pool.tile([128, TW], BF16, tag="tmp" + str(k))
                nc.vector.tensor_sub(tk[:, :], src[:, TW * k:TW * k + TW], mu[:, :])
                tks.append(tk)
            nc.vector.reciprocal(rst[:, :], std[:, :])
            nc.vector.tensor_copy(rstb[:, :], rst[:, :])
            for k in range(3):
                tk = tks[k]
                nc.vector.tensor_mul(tk[:, :], tk[:, :], rstb[:, :])
                nc.vector.tensor_scalar(tk[:, :], tk[:, :], g_sb[:, k:k + 1],
                                        be_sb[:, k:k + 1], op0=OP.mult, op1=OP.add)
                if not wrap_fix:
                    nc.vector.tensor_add(dst[:, TW * k:TW * k + TW], tk[:, :],
                                         skip[:, TW * k:TW * k + TW])
                    continue
                # main run stays (qr, qc)-major; the wrap window's 49 columns
                # are written qc-major so T-out/scatter see contiguous
                # partition runs.
                nc.vector.tensor_add(dst[:, TW * k:TW * k + 343], tk[:, 0:343],
                                     skip[:, TW * k:TW * k + 343])
                ptk = tk[:, :].ap[0][0]
                tkoff = tk[:, :].offset
                psk = skip[:, :].ap[0][0]
                skoff = skip[:, :].offset
                pdst = dst[:, :].ap[0][0]
                dstoff = dst[:, :].offset
                nc.vector.tensor_add(
                    _ap(dst, dstoff + TW * k + 343, [[pdst, 128], [WS, WS], [1, WS]]),
                    _ap(tk, tkoff + 343, [[ptk, 128], [1, WS], [WS, WS]]),
                    _ap(skip, skoff + TW * k + 343, [[psk, 128], [1, WS], [WS, WS]]))

        # ---------- main loop: 4 images x 8 bands, software-pipelined ----------
        # Emission order per step i:  gather(i+2) | B(i)=out-proj+LN1 |
        # A(i+1)=T-in+v-proj+permute+fold | D(i-1)=T-out+scatter | C(i)=MLP+LN2.
        # The A(i+1) PE work covers LN1(i)'s DVE latency; D(i-1) runs after
        # LN2(i-1) has long finished; B(i)'s out-proj finds xcm(i) ready.
        BANDS = [(img, band) for img in range(IMGS) for band in range(NWS)]
        state = {}

        def gather(i):
            img, band = BANDS[i]
            stage = stage_pool.tile([98, 4 * C], F32, tag="stage")
            _band_io_dmas(x_d, img, band, stage, stage[:, :].offset,
                          stage[:, :].ap[0][0], gather=True,
                          dma_fn=nc.sync.dma_start)
            state[("stage", i)] = stage

        def stage_A(i):
            """T-in -> xt; v-projection -> vt; permute -> ld; fold -> xcm."""
            stage = state.pop(("stage", i))
            xt = xt_pool.tile([128, 3 * TW], BF16, tag="xt")
            for k in range(3):
                xps = ps_t.tile([128, TW], F32, tag="t")
                for wp in range(4):
                    nc.tensor.transpose(
                        xps[:, 98 * wp:98 * wp + 98],
                        stage[0:98, wp * C + 128 * k:wp * C + 128 * k + 128],
                        ident[0:98, 0:98])
                pxp = xps[:, :].ap[0][0]
                xpoff = xps[:, :].offset
                pxt = xt[:, :].ap[0][0]
                xtoff = xt[:, :].offset
                nc.scalar.copy(xt[:, TW * k:TW * k + 343], xps[:, 0:343])
                nc.scalar.copy(
                    _ap(xt, xtoff + TW * k + 343, [[pxt, 128], [WS, WS], [1, WS]]),
                    _ap(xps, xpoff + 343, [[pxp, 128], [1, WS], [WS, WS]]))
            vt = vt_pool.tile([128, 3 * TW], BF16, tag="vt")
            for kv in range(3):
                vps = ps_big.tile([128, TW], F32, tag="big")
                for k in range(3):
                    nc.tensor.matmul(vps[:, :],
                                     wv_r[:, C * k + 128 * kv:C * k + 128 * kv + 128],
                                     xt[:, TW * k:TW * k + TW],
                                     start=(k == 0), stop=(k == 2))
                nc.scalar.activation(vt[:, TW * kv:TW * kv + TW], vps[:, :],
                                     AF.Identity, bias=bv_sb[:, kv:kv + 1], scale=1.0)
            ld = ld_pool.tile([32, WT * J], BF16, tag="ld")
            pld = ld[:, :].ap[0][0]
            ldoff = ld[:, :].offset
            pvt = vt[:, :].ap[0][0]
            vtoff = vt[:, :].offset
            for h in range(12):
                src = _ap(vt, vtoff + ((h % 4) * 32) * pvt + (h // 4) * TW,
                          [[pvt, 32], [1, TW]])
                dst = _ap(ld, ldoff + N * h,
                          [[pld, 32], [J, WT], [1, N]])
                if h % 4 == 0:
                    nc.scalar.dma_start(dst, src)
                elif h % 4 == 2:
                    nc.sync.dma_start(dst, src)
                else:
                    nc.gpsimd.dma_start(dst, src)
            state[("xt", i)] = xt
            state[("ld", i)] = ld

        def stage_A2(i):
            """Fold L_d -> channel-major xcm (strided copies, DVE/Pool)."""
            ld = state.pop(("ld", i))
            pld = ld[:, :].ap[0][0]
            ldoff = ld[:, :].offset
            xcm = xc_pool.tile([128, 3 * TW], BF16, tag="xcm")
            pxc = xcm[:, :].ap[0][0]
            xcoff = xcm[:, :].offset
            for kj in range(3):
                for rr in range(4):
                    csrc = _ap(ld, ldoff + 4 * kj + rr,
                               [[pld, 32], [J, WT], [12, N]])
                    cdst = _ap(xcm, xcoff + 32 * rr * pxc + TW * kj,
                               [[pxc, 32], [1, TW]])
                    nc.gpsimd.tensor_copy(cdst, csrc)
            state[("xcm", i)] = xcm

        def stage_B_out(i):
            """out-projection (K=128) -> y."""
            xcm = state.pop(("xcm", i))
            y = y_pool.tile([128, 3 * TW], BF16, tag="y")
            for kj in range(3):
                yps = ps_big.tile([128, TW], F32, tag="big")
                for k in range(3):
                    nc.tensor.matmul(yps[:, :],
                                     wo_r[:, C * k + 128 * kj:C * k + 128 * kj + 128],
                                     xcm[:, TW * k:TW * k + TW],
                                     start=(k == 0), stop=(k == 2))
                nc.vector.tensor_scalar(y[:, TW * kj:TW * kj + TW], yps[:, :],
                                        1.0, bo_sb[:, kj:kj + 1],
                                        op0=OP.mult, op1=OP.add)
            state[("y", i)] = y

        def stage_B_ln_pre(i):
            state[("ln1", i)] = layernorm_pre(state.pop(("y", i)), g1_sb, be1_sb)

        def stage_B_ln_post(i):
            var, tks = state.pop(("ln1", i))
            x2 = x2_pool.tile([128, 3 * TW], BF16, tag="x2")
            layernorm_post(var, tks, x2, state.pop(("xt", i)), g1_sb, be1_sb)
            state[("x2", i)] = x2

        def stage_C(i):
            """MLP (gelu) and LayerNorm2 + skip2 -> ocm."""
            x2 = state.pop(("x2", i))
            hsb = h_pool.tile([128, 12 * TW], BF16, tag="h")
            for m in range(12):
                hps = ps_big.tile([128, TW], F32, tag="big")
                for k in range(3):
                    nc.tensor.matmul(hps[:, :],
                                     w1_r[:, MLP * k + 128 * m:MLP * k + 128 * m + 128],
                                     x2[:, TW * k:TW * k + TW],
                                     start=(k == 0), stop=(k == 2))
                nc.scalar.activation(hsb[:, TW * m:TW * m + TW], hps[:, :],
                                     AF.Gelu, bias=b1_sb[:, m:m + 1], scale=1.0)
            h2 = y_pool.tile([128, 3 * TW], BF16, tag="h2")
            for kj in range(3):
                h2ps = ps_big.tile([128, TW], F32, tag="big")
                for k2 in range(12):
                    nc.tensor.matmul(h2ps[:, :],
                                     w2_r[:, C * k2 + 128 * kj:C * k2 + 128 * kj + 128],
                                     hsb[:, TW * k2:TW * k2 + TW],
                                     start=(k2 == 0), stop=(k2 == 11))
                nc.vector.tensor_scalar(h2[:, TW * kj:TW * kj + TW], h2ps[:, :],
                                        1.0, b2_sb[:, kj:kj + 1],
                                        op0=OP.mult, op1=OP.add)
            ocm = oc_pool.tile([128, 3 * TW], BF16, tag="oc")
            layernorm(h2, ocm, x2, g2_sb, be2_sb, wrap_fix=True)
            state[("oc", i)] = ocm

        def stage_D(i):
            """Transpose back and scatter the band."""
            img, band = BANDS[i]
            ocm = state.pop(("oc", i))
            otm = ot_pool.tile([98, 4 * C], F32, tag="ot")
            for wp in range(4):
                otps = ps_tb.tile([98, 3 * 128], BF16, tag="tb")
                for k in range(3):
                    nc.tensor.transpose(
                        otps[0:98, 128 * k:128 * k + 128],
                        ocm[:, TW * k + 98 * wp:TW * k + 98 * wp + 98],
                        ident_b[:, :])
                nc.scalar.copy(otm[0:98, wp * C:wp * C + C],
                               otps[0:98, 0:C])
            _band_io_dmas(out_d, img, band, otm, otm[:, :].offset,
                          otm[:, :].ap[0][0], gather=False,
                          dma_fn=nc.sync.dma_start)

        NB = len(BANDS)
        gather(0)
        gather(1)
        stage_A(0)
        stage_A2(0)
        for i in range(NB):
            if i + 1 < NB:
                gather(i + 1)
            stage_B_out(i)
            if i + 1 < NB:
                stage_A(i + 1)
                stage_A2(i + 1)
            if i >= 2:
                stage_D(i - 2)
            stage_B_ln_pre(i)
            stage_B_ln_post(i)
            if i >= 1:
                stage_C(i - 1)
        stage_C(NB - 1)
        stage_D(NB - 2)
        stage_D(NB - 1)
    nc.compile()
    return nc


_CACHE = {}


def kernel(**inputs):
    if "nc" not in _CACHE:
        _CACHE["nc"] = build()
    nc = _CACHE["nc"]
    x = np.ascontiguousarray(np.asarray(inputs["x"], dtype=np.float32))
    base = {n: np.ascontiguousarray(np.asarray(inputs[n], dtype=np.float32))
            for n in WNAMES}
    in_maps = []
    for c in range(NCORES):
        m = dict(base)
        m["x"] = np.ascontiguousarray(x[IMGS * c:IMGS * (c + 1)])
        in_maps.append(m)
    import os
    trace = bool(int(os.environ.get("KERNEL_TRACE", "0")))
    res = run_bass_kernel_spmd(nc, in_maps, core_ids=list(range(NCORES)),
                               trace=trace)
    _CACHE["last_res"] = res
    out = np.concatenate([r["out"] for r in res.results], axis=0)
    return out
